# revision 1
# baseline (speedup 1.0000x reference)
"""BandSplit (BSRNN) Trainium2 kernel.

Math per band k (31 bands over 257 freq bins, groups of band width 3/6/16/27):
  xg = x[b, :, band_bins, t] flattened to d = 2*bw features (torch order:
       bin-major, re/im minor)
  out[b, k, t, :] = LayerNorm_d(xg) @ W_k + b_k          (d -> C=128)

Algebraic refactor used here (per band, per t):
  mu    = mean_d(x),  var = mean_d(x^2) - mu^2,  rstd = rsqrt(var + eps)
  out   = rstd*(x @ Wg) - (mu*rstd)*(u) + bb
  with (host-precomputed)  Wg = gamma*W,  u = sum_d Wg,  bb = b + beta @ W.
So on device, for pack lhsT rows = [x*rstd_rep ; pad ; mu*rstd ; ones ; pad]
and rhs = [Wg blockdiag ; 0 ; -u blockdiag ; bb row ; 0], a single fp32
matmul per (pack, t-chunk) emits the FINAL output tile (t x (n*128)) in PSUM.

Sharding: batch-parallel, core b handles x[b] (B=8 = n_cores).
"""

import numpy as np

T = 3000
C = 128
F_BINS = 257
EPS = 1e-5
GROUPS = [(10, 3), (12, 6), (8, 16), (1, 27)]  # (n_bands, bins_per_band)

SPAN = 512   # stats/prep span (free dim of PSUM bank)
CHUNK = 128  # output t-chunk (PSUM partition dim)


# ---------------------------------------------------------------- metadata --
class Band:
    def __init__(self, g, i, f0, bw):
        self.g, self.i, self.f0, self.bw = g, i, f0, bw


BANDS = []
_f0 = 0
for _g, (_n, _bw) in enumerate(GROUPS):
    for _i in range(_n):
        BANDS.append(Band(_g, _i, _f0, _bw))
        _f0 += _bw
assert _f0 == F_BINS and len(BANDS) == 31


class Pack:
    def __init__(self, pid, band_ids, qset, quad):
        self.pid = pid
        self.band_ids = list(band_ids)
        self.n = len(self.band_ids)
        self.bw = BANDS[self.band_ids[0]].bw
        self.d = 2 * self.bw
        self.F = self.n * self.d                    # feature rows
        self.F32 = ((self.F + 31) // 32) * 32       # aligned start of mu'' rows
        self.K = self.F32 + 32                      # lhsT partition count
        self.row_ones = self.F32 + self.n
        self.k0 = self.band_ids[0]                  # first global band
        self.f0 = BANDS[self.k0].f0                 # first freq bin
        self.qset = qset                            # 'A' or 'B'
        self.quad = quad                            # quadrant row base /32
        assert self.K <= 128 and self.F32 + self.n + 1 <= self.K


# matmul tile_position bases are limited to {0, 32, 64} (quadrant-3 HW bug),
# so at most 3 packs share a stats/srstd tile set.
PACKS = [
    Pack(0, range(0, 10), 'A', 0),
    Pack(1, range(10, 16), 'A', 1),
    Pack(2, range(16, 22), 'A', 2),
    Pack(3, range(22, 25), 'B', 0),
    Pack(4, range(25, 28), 'B', 1),
    Pack(5, range(28, 30), 'B', 2),
    Pack(6, range(30, 31), 'C', 0),
]
QSETS = "ABC"
EREP_COLS = max(p.F for p in PACKS)  # 96

SPANS = [(s0, min(SPAN, T - s0)) for s0 in range(0, T, SPAN)]


def _blocks(p):
    """512-wide column blocks of pack p's (n*128) output."""
    out = []
    for n0 in range(0, p.n * C, 512):
        nw = min(512, p.n * C - n0)
        out.append((n0, nw, p.k0 + n0 // C, nw // C))  # col0, width, band0, nbands
    return out


# ------------------------------------------------------------- host params --
def _host_params(inputs):
    f32 = np.float32
    ext = {}
    for p in PACKS:
        wext = np.zeros((p.K, p.n * C), f32)
        selx = np.zeros((p.F, 32), f32)  # 32 cols: full quadrant write
        F2 = p.F // 2
        for j, k in enumerate(p.band_ids):
            b = BANDS[k]
            W = np.asarray(inputs[f"g{b.g}_W"][b.i], f32)        # (d, C)
            gam = np.asarray(inputs[f"g{b.g}_gamma"][b.i], f32)  # (d,)
            bet = np.asarray(inputs[f"g{b.g}_beta"][b.i], f32)
            bias = np.asarray(inputs[f"g{b.g}_b"][b.i], f32)     # (C,)
            Wg = gam[:, None] * W
            cols = slice(j * C, (j + 1) * C)
            # device row layout is plane-major: row pl*F2 + j*bw + f
            # holds (plane pl, bin f) of band j == torch feature 2f+pl
            for pl in range(2):
                rows = slice(pl * F2 + j * p.bw, pl * F2 + (j + 1) * p.bw)
                wext[rows, cols] = Wg[2 * np.arange(p.bw) + pl]
                selx[rows, j] = 1.0 / p.d
            wext[p.F32 + j, cols] = -Wg.sum(0)
            wext[p.row_ones, cols] = bias + bet @ W
        ext[f"wext{p.pid}"] = wext
        ext[f"selx{p.pid}"] = selx
    for qs in QSETS:
        erep = np.zeros((128, EREP_COLS), f32)
        for p in PACKS:
            if p.qset != qs:
                continue
            F2 = p.F // 2
            for j in range(p.n):
                for pl in range(2):
                    r0 = pl * F2 + j * p.bw
                    erep[32 * p.quad + j, r0:r0 + p.bw] = 1.0
        ext[f"erep{qs}"] = erep
    return ext


# ------------------------------------------------------------ device build --
_CACHE = {}


def _build():
    if "nc" in _CACHE:
        return _CACHE["nc"]
    import concourse.bacc as bacc
    import concourse.tile as tile
    from concourse import mybir

    Alu = mybir.AluOpType
    Act = mybir.ActivationFunctionType
    F32 = mybir.dt.float32
    F32R = mybir.dt.float32r

    nc = bacc.Bacc("TRN2", target_bir_lowering=False, debug=False, num_devices=8)
    x_d = nc.dram_tensor("x", [2 * F_BINS, T], F32, kind="ExternalInput")
    out_d = nc.dram_tensor("out", [31, T, C], F32, kind="ExternalOutput")
    wext_d = {p.pid: nc.dram_tensor(f"wext{p.pid}", [p.K, p.n * C], F32,
                                    kind="ExternalInput") for p in PACKS}
    selx_d = {p.pid: nc.dram_tensor(f"selx{p.pid}", [p.F, 32], F32,
                                    kind="ExternalInput") for p in PACKS}
    erep_d = {qs: nc.dram_tensor(f"erep{qs}", [128, EREP_COLS], F32,
                                 kind="ExternalInput") for qs in QSETS}

    with tile.TileContext(nc) as tc:
        with (
            tc.tile_pool(name="const", bufs=1) as const,
            tc.tile_pool(name="xsqp", bufs=2) as xsqp,
            tc.tile_pool(name="stt", bufs=2) as stt,
            tc.tile_pool(name="obp", bufs=4) as obp,
            tc.tile_pool(name="srp", bufs=2) as srp,
            tc.tile_pool(name="stps", bufs=1, space="PSUM") as stps,
            tc.tile_pool(name="rrps", bufs=1, space="PSUM") as rrps,
            tc.tile_pool(name="outps", bufs=3, space="PSUM") as outps,
        ):
            # ---- resident constants
            xin = {}
            wext = {}
            selx = {}
            for p in PACKS:
                xin[p.pid] = const.tile([p.F, T], F32, tag=f"xin{p.pid}", name=f"xin{p.pid}")
                # SBUF rows are plane-major: [all re bins | all im bins],
                # so each plane is one contiguous 2D DMA.
                F2 = p.F // 2
                for pl in range(2):
                    s_ = x_d[pl * F_BINS + p.f0:
                             pl * F_BINS + p.f0 + F2, :]
                    d_ = xin[p.pid][pl * F2:(pl + 1) * F2, :]
                    eng = nc.sync if (p.pid + pl) % 2 == 0 else nc.scalar
                    eng.dma_start(out=d_, in_=s_)
                wext[p.pid] = const.tile([p.K, p.n * C], F32, tag=f"wx{p.pid}", name=f"wx{p.pid}")
                nc.scalar.dma_start(out=wext[p.pid][:], in_=wext_d[p.pid][:])
                selx[p.pid] = const.tile([p.F, 32], F32, tag=f"sx{p.pid}", name=f"sx{p.pid}")
                nc.sync.dma_start(out=selx[p.pid][:], in_=selx_d[p.pid][:])
            erep = {}
            for qs in QSETS:
                erep[qs] = const.tile([128, EREP_COLS], F32, tag=f"er{qs}", name=f"er{qs}")
                nc.sync.dma_start(out=erep[qs][:], in_=erep_d[qs][:])

            eps_t = const.tile([128, 1], F32, tag="epsc", name="epsc")
            nc.vector.memset(eps_t[:], EPS)

            # persistent double-buffered lhsT tiles; pad/ones rows are
            # written once (mu''/xs rows get rewritten every span).
            # memset can't emit fp32r, so fill via an fp32 scratch + copy.
            ones_t = const.tile([64, SPAN], F32, tag="ones", name="ones")
            nc.gpsimd.memset(ones_t[:], 1.0)
            xpt = {}
            for p in PACKS:
                for par in range(2):
                    t_ = const.tile([p.K, SPAN], F32, tag=f"xp{p.pid}_{par}",
                                    name=f"xp{p.pid}_{par}")
                    xpt[(p.pid, par)] = t_
                    for m0 in range(self_ms(p), p.K, 32):
                        nc.vector.tensor_copy(
                            t_[m0:m0 + 32, :], ones_t[0:32, :])

            drain_i = 0
            for (s0, sw) in SPANS:
                # ---- A) per-band sums via PE: mu / meansq at quadrant rows
                mu = {qs: stps.tile([128, SPAN], F32, tag="mu", name=f"mu{qs}", bufs=2) for qs in "AB"}
                msq = {qs: stps.tile([128, SPAN], F32, tag="msq", name=f"ms{qs}", bufs=2) for qs in "AB"}
                # C set (P6 only): mu at row 0, meansq at row 32 of one bank
                muc = stps.tile([64, SPAN], F32, tag="mu", name="muC", bufs=2)
                srstd = {qs: srp.tile([128, SPAN], F32, tag=f"sr{qs}",
                                      name=f"sr{qs}") for qs in QSETS}

                def mu_ap(p, q0, w=None):
                    w = p.n if w is None else w
                    if p.qset == "C":
                        return muc[0:w, :sw]
                    return mu[p.qset][q0:q0 + w, :sw]

                def msq_ap(p, q0, w=None):
                    w = p.n if w is None else w
                    if p.qset == "C":
                        return muc[32:32 + w, :sw]
                    return msq[p.qset][q0:q0 + w, :sw]

                for p in PACKS:
                    q0 = 32 * p.quad
                    xsq = xsqp.tile([128, SPAN], F32, tag="xsq", name="xsq")
                    xin_f = xin[p.pid][:, s0:s0 + sw]
                    nc.gpsimd.tensor_tensor(
                        xsq[0:p.F, :sw], xin_f, xin_f, op=Alu.mult)
                    nc.tensor.matmul(
                        mu_ap(p, q0, 32), selx[p.pid][:],
                        xin[p.pid][:, s0:s0 + sw], start=True, stop=True)
                    nc.tensor.matmul(
                        msq_ap(p, q0, 32), selx[p.pid][:],
                        xsq[0:p.F, :sw], start=True, stop=True)
                # ---- B) rstd = rsqrt(msq - mu^2 + eps), batched per set
                for qs in "AB":
                    musq = stt.tile([128, SPAN], F32, tag="musq", name="musq")
                    nc.scalar.activation(
                        musq[0:96, :sw], mu[qs][0:96, :sw], Act.Square)
                    var = stt.tile([128, SPAN], F32, tag="var", name="var")
                    nc.vector.tensor_tensor(
                        var[0:96, :sw], msq[qs][0:96, :sw],
                        musq[0:96, :sw], op=Alu.subtract)
                    sq = stt.tile([128, SPAN], F32, tag="sq", name="sq")
                    nc.scalar.activation(
                        sq[0:96, :sw], var[0:96, :sw], Act.Sqrt,
                        bias=eps_t[0:96, 0:1], scale=1.0)
                    scr = stt.tile([128, SPAN], F32, tag="scr", name="scr")
                    nc.vector.reciprocal_approx_accurate(
                        out=srstd[qs][0:96, :sw], in_=sq[0:96, :sw],
                        scratch=scr[0:96, :sw])
                # C set scalar path (1 band)
                musc = stt.tile([128, SPAN], F32, tag="musq", name="musc")
                nc.scalar.activation(
                    musc[0:1, :sw], muc[0:1, :sw], Act.Square)
                varc = stt.tile([128, SPAN], F32, tag="var", name="varc")
                nc.vector.tensor_tensor(
                    varc[0:1, :sw], muc[32:33, :sw], musc[0:1, :sw],
                    op=Alu.subtract)
                sqc = stt.tile([128, SPAN], F32, tag="sq", name="sqc")
                nc.scalar.activation(
                    sqc[0:1, :sw], varc[0:1, :sw], Act.Sqrt,
                    bias=eps_t[0:1, 0:1], scale=1.0)
                scrc = stt.tile([128, SPAN], F32, tag="scr", name="scrc")
                nc.vector.reciprocal_approx_accurate(
                    out=srstd["C"][0:1, :sw], in_=sqc[0:1, :sw],
                    scratch=scrc[0:1, :sw])
                # ---- C) pack lhsT prep: [x*rstd_rep ; mu*rstd ; ones]
                xp = {}
                for p in PACKS:
                    q0 = 32 * p.quad
                    t_ = xpt[(p.pid, (s0 // SPAN) % 2)]
                    xp[p.pid] = t_
                    nc.vector.tensor_tensor(
                        t_[p.F32:p.F32 + p.n, :sw], mu_ap(p, q0),
                        srstd[p.qset][q0:q0 + p.n, :sw], op=Alu.mult)
                    rr = rrps.tile([128, SPAN], F32, tag="rr", name="rr")
                    nc.tensor.matmul(
                        rr[0:p.F, :sw],
                        erep[p.qset][q0:q0 + p.n, 0:p.F],
                        srstd[p.qset][q0:q0 + p.n, :sw],
                        start=True, stop=True)
                    nc.vector.tensor_tensor(
                        t_[0:p.F, :sw], xin[p.pid][:, s0:s0 + sw],
                        rr[0:p.F, :sw], op=Alu.mult)
                # ---- D) main matmuls + drain + store per t-chunk
                for c0 in range(s0, s0 + sw, CHUNK):
                    cw = min(CHUNK, s0 + sw - c0)
                    for p in PACKS:
                        lhsT = xp[p.pid][0:p.K, c0 - s0:c0 - s0 + cw]
                        ob = obp.tile([128, 1280], F32, tag="ob", name="ob")
                        for (n0, nw, kb0, nb) in _blocks(p):
                            op = outps.tile([128, 512], F32, tag="op", name="op")
                            nc.tensor.matmul(
                                op[0:cw, 0:nw], lhsT,
                                wext[p.pid][:, n0:n0 + nw],
                                start=True, stop=True)
                            if drain_i % 4 < 3:
                                nc.scalar.activation(
                                    ob[0:cw, n0:n0 + nw], op[0:cw, 0:nw],
                                    Act.Copy)
                            else:
                                nc.vector.tensor_copy(
                                    ob[0:cw, n0:n0 + nw], op[0:cw, 0:nw])
                            drain_i += 1
                        dst = out_d[p.k0:p.k0 + p.n, c0:c0 + cw, :]
                        dst = dst.rearrange("j t c -> t j c")
                        src = ob[0:cw, 0:p.n * C].rearrange(
                            "t (j c) -> t j c", c=C)
                        eng = (nc.sync, nc.scalar)[drain_i % 2]
                        eng.dma_start(out=dst, in_=src)

    nc.compile()
    _CACHE["nc"] = nc
    return nc


def self_ms(p):
    """memset start row: covers [F, K) zero/one padding, 32-aligned."""
    return max(0, p.F32 - 32)


# ------------------------------------------------------------------ driver --
def kernel(**inputs):
    from concourse.bass_utils import run_bass_kernel_spmd

    x = np.ascontiguousarray(np.asarray(inputs["x"], np.float32))
    B = x.shape[0]
    assert x.shape == (8, 2, F_BINS, T)
    ext = _host_params(inputs)
    nc = _build()
    in_maps = []
    for b in range(B):
        m = {"x": x[b].reshape(2 * F_BINS, T)}
        m.update(ext)
        in_maps.append(m)
    res = run_bass_kernel_spmd(nc, in_maps, core_ids=list(range(8)))
    out = np.stack([res.results[b]["out"] for b in range(B)], axis=0)
    return out.astype(np.float32, copy=False)



# revision 43
# speedup vs baseline: 1.4732x; 1.4732x over previous
"""BandSplit (BSRNN) Trainium2 kernel.

Math per band k (31 bands over 257 freq bins, groups of band width 3/6/16/27):
  xg = x[b, :, band_bins, t] flattened to d = 2*bw features (torch order:
       bin-major, re/im minor)
  out[b, k, t, :] = LayerNorm_d(xg) @ W_k + b_k          (d -> C=128)

Algebraic refactor used here (per band, per t):
  mu    = mean_d(x),  var = mean_d(x^2) - mu^2,  rstd = rsqrt(var + eps)
  out   = rstd*(x @ Wg) - (mu*rstd)*(u) + bb
  with (host-precomputed)  Wg = gamma*W,  u = sum_d Wg,  bb = b + beta @ W.
So on device, for pack lhsT rows = [x*rstd_rep ; pad ; mu*rstd ; ones ; pad]
and rhs = [Wg blockdiag ; 0 ; -u blockdiag ; bb row ; 0], a single fp32r
matmul per (pack, t-chunk) emits the FINAL output tile (t x (n*128)) in PSUM.

All matmul operands are bitcast to float32r: same fp32 bytes, but the PE
streams 1 column/cycle (vs 4 for plain fp32) when the moving free dim >= 256.

Per 128-t chunk the 11 block PSUM tiles drain (Act/DVE/Pool rotation) into one
wide SBUF tile [128 x 31*128], shipped to DRAM by a single DMA (t-major, one
512B descriptor per (t, band) run) -- full 360 GB/s with ~1 HWDGE setup/chunk.

Pipeline: span s+1's stats/rstd/lhsT prep is issued between chunk 0 and
chunk 1 of span s, so per-engine program order never stalls the drain +
store stream at span boundaries. All input DMAs ride the SP queue in
consumption order (pack by pack, erep as soon as its packs are present);
output DMAs follow on the same queue.

Sharding: batch-parallel, core b handles x[b] (B=8 = n_cores).
"""

import numpy as np

T = 3000
C = 128
F_BINS = 257
EPS = 1e-5
GROUPS = [(10, 3), (12, 6), (8, 16), (1, 27)]  # (n_bands, bins_per_band)

SPAN = 512   # stats/prep span (free dim of PSUM bank)
CHUNK = 128  # output t-chunk (PSUM partition dim)
OBCOLS = 31 * C


# ---------------------------------------------------------------- metadata --
class Band:
    def __init__(self, g, i, f0, bw):
        self.g, self.i, self.f0, self.bw = g, i, f0, bw


BANDS = []
_f0 = 0
for _g, (_n, _bw) in enumerate(GROUPS):
    for _i in range(_n):
        BANDS.append(Band(_g, _i, _f0, _bw))
        _f0 += _bw
assert _f0 == F_BINS and len(BANDS) == 31


class Pack:
    def __init__(self, pid, band_ids, qset, quad):
        self.pid = pid
        self.band_ids = list(band_ids)
        self.n = len(self.band_ids)
        self.bw = BANDS[self.band_ids[0]].bw
        self.d = 2 * self.bw
        self.F = self.n * self.d                    # feature rows
        self.F32 = ((self.F + 31) // 32) * 32       # aligned start of mu'' rows
        self.K = self.F32 + 32                      # lhsT partition count
        self.row_ones = self.F32 + self.n
        self.k0 = self.band_ids[0]                  # first global band
        self.f0 = BANDS[self.k0].f0                 # first freq bin
        self.qset = qset                            # 'A' or 'B'
        self.quad = quad                            # quadrant row base /32
        assert self.K <= 128 and self.F32 + self.n + 1 <= self.K


# matmul tile_position bases are limited to {0, 32, 64} (quadrant-3 HW bug),
# so at most 3 packs share a stats/srstd tile set.
PACKS = [
    Pack(0, range(0, 10), 'A', 0),
    Pack(1, range(10, 16), 'A', 1),
    Pack(2, range(16, 22), 'A', 2),
    Pack(3, range(22, 25), 'B', 0),
    Pack(4, range(25, 28), 'B', 1),
    Pack(5, range(28, 30), 'B', 2),
    Pack(6, range(30, 31), 'C', 0),
]
QSETS = "ABC"
EREP_COLS = max(p.F for p in PACKS)  # 96
# emission / load order: C first (one tiny band -> its chain clears the
# in-order Act/DVE queues almost immediately), then A, then B.
SET_ORDER = ["C", "A", "B"]
LOAD_ORDER = [6, 0, 1, 2, 3, 4, 5]

SPANS = [(s0, min(SPAN, T - s0)) for s0 in range(0, T, SPAN)]

# drain engine per block, rotating: a=Act (fastest), v=DVE, p=Pool.
# 11 blocks/chunk with widths [512,512,256, 512,256, 512,256, 384, 384,
# 256, 128]; pattern balances per-chunk drain time across the three.
DRAIN_PAT = ["a", "v", "a", "a", "v", "a", "a", "a", "a", "v", "a"]


def _blocks(p):
    """512-wide column blocks of pack p's (n*128) output."""
    out = []
    for n0 in range(0, p.n * C, 512):
        nw = min(512, p.n * C - n0)
        out.append((n0, nw, p.k0 + n0 // C, nw // C))  # col0, width, band0, nbands
    return out


# ------------------------------------------------------------- host params --
def _host_params(inputs):
    f32 = np.float32
    ext = {}
    for p in PACKS:
        # selx rides as 32 extra columns of wext (rows 0:F) so one DMA
        # loads both; wext pad rows [F,F32) and [row_ones+1,K) stay home.
        wext = np.zeros((p.K, p.n * C + 32), f32)
        selx = wext[0:p.F, p.n * C:]  # view: full quadrant write
        F2 = p.F // 2
        for j, k in enumerate(p.band_ids):
            b = BANDS[k]
            W = np.asarray(inputs[f"g{b.g}_W"][b.i], f32)        # (d, C)
            gam = np.asarray(inputs[f"g{b.g}_gamma"][b.i], f32)  # (d,)
            bet = np.asarray(inputs[f"g{b.g}_beta"][b.i], f32)
            bias = np.asarray(inputs[f"g{b.g}_b"][b.i], f32)     # (C,)
            Wg = gam[:, None] * W
            cols = slice(j * C, (j + 1) * C)
            # device row layout is plane-major: row pl*F2 + j*bw + f
            # holds (plane pl, bin f) of band j == torch feature 2f+pl
            for pl in range(2):
                rows = slice(pl * F2 + j * p.bw, pl * F2 + (j + 1) * p.bw)
                wext[rows, cols] = Wg[2 * np.arange(p.bw) + pl]
                selx[rows, j] = 1.0 / p.d
            wext[p.F32 + j, cols] = -Wg.sum(0)
            wext[p.row_ones, cols] = bias + bet @ W
        ext[f"wext{p.pid}"] = wext
    for qs in QSETS:
        erep = np.zeros((128, EREP_COLS), f32)
        for p in PACKS:
            if p.qset != qs:
                continue
            F2 = p.F // 2
            for j in range(p.n):
                for pl in range(2):
                    r0 = pl * F2 + j * p.bw
                    erep[32 * p.quad + j, r0:r0 + p.bw] = 1.0
        ext[f"erep{qs}"] = erep
    return ext


# ------------------------------------------------------------ device build --
_CACHE = {}


def _build():
    if "nc" in _CACHE:
        return _CACHE["nc"]
    import concourse.bacc as bacc
    import concourse.tile as tile
    from concourse import mybir

    Alu = mybir.AluOpType
    Act = mybir.ActivationFunctionType
    F32 = mybir.dt.float32
    F32R = mybir.dt.float32r

    nc = bacc.Bacc("TRN2", target_bir_lowering=False, debug=False, num_devices=8)
    x_d = nc.dram_tensor("x", [2 * F_BINS, T], F32R, kind="ExternalInput")
    out_d = nc.dram_tensor("out", [31, T, C], F32, kind="ExternalOutput")
    wext_d = {p.pid: nc.dram_tensor(f"wext{p.pid}", [p.K, p.n * C + 32], F32R,
                                    kind="ExternalInput") for p in PACKS}
    erep_d = {qs: nc.dram_tensor(f"erep{qs}", [128, EREP_COLS], F32,
                                 kind="ExternalInput") for qs in QSETS}

    with tile.TileContext(nc) as tc:
        with (
            tc.tile_pool(name="const", bufs=1) as const,
            tc.tile_pool(name="xsqp", bufs=2) as xsqp,
            tc.tile_pool(name="stt", bufs=2) as stt,
            tc.tile_pool(name="obp", bufs=2) as obp,
            tc.tile_pool(name="srp", bufs=1) as srp,
            tc.tile_pool(name="stps", bufs=1, space="PSUM") as stps,
            tc.tile_pool(name="rrps", bufs=1, space="PSUM") as rrps,
            tc.tile_pool(name="outps", bufs=4, space="PSUM") as outps,
        ):
            # ---- resident constants, loaded on the SP queue in the order
            # the compute pipeline consumes them (pack by pack; erep for a
            # qset as soon as all its packs are in flight).
            xin_t = {}
            wext = {}
            selx = {}
            erep = {}
            for p in PACKS:
                xin_t[p.pid] = const.tile([p.F, T], F32R, tag=f"xin{p.pid}", name=f"xin{p.pid}")
                wext[p.pid] = const.tile([p.K, p.n * C + 32], F32R, tag=f"wx{p.pid}", name=f"wx{p.pid}")
                selx[p.pid] = wext[p.pid][0:p.F, p.n * C:p.n * C + 32]
            for qs in QSETS:
                erep[qs] = const.tile([128, EREP_COLS], F32, tag=f"er{qs}", name=f"er{qs}")

            def xin(pid, s0, sw):
                return xin_t[pid][:, s0:s0 + sw]

            for pid in LOAD_ORDER:
                p = PACKS[pid]
                F2 = p.F // 2
                # SBUF rows are plane-major: [all re bins | all im bins],
                # one contiguous 2D DMA per plane.
                for pl in range(2):
                    s_ = x_d[pl * F_BINS + p.f0:pl * F_BINS + p.f0 + F2, :]
                    d_ = xin_t[p.pid][pl * F2:(pl + 1) * F2, :]
                    nc.sync.dma_start(out=d_, in_=s_)
                # full K rows: the all-zero pad rows must be resident so
                # lhsT pad rows (set to 1.0) multiply against true zeros
                nc.sync.dma_start(out=wext[p.pid][:], in_=wext_d[p.pid][:])
                qs = p.qset
                if pid == max(q.pid for q in PACKS if q.qset == qs):
                    nc.sync.dma_start(out=erep[qs][:], in_=erep_d[qs][:])

            eps_t = const.tile([128, 1], F32, tag="epsc", name="epsc")
            nc.vector.memset(eps_t[:], EPS)

            # persistent double-buffered lhsT tiles (f32r). Rows
            # [floor32(F), K) are filled with 1.0 once: the ones row gets
            # its 1.0, pad rows become finite don't-cares (their wext rows
            # are zero), data rows in range are rewritten every span.
            # memset can't emit fp32r, so fill via fp32 scratch + DVE copy
            # (the copy rounds on write).
            ones_t = const.tile([32, SPAN], F32, tag="ones", name="ones")
            nc.gpsimd.memset(ones_t[:], 1.0)
            xpt = {}
            for p in PACKS:
                ms0 = (p.F // 32) * 32
                for par in range(2):
                    t_ = const.tile([p.K, SPAN], F32R, tag=f"xp{p.pid}_{par}",
                                    name=f"xp{p.pid}_{par}")
                    xpt[(p.pid, par)] = t_
                    for m0 in range(ms0, p.K, 32):
                        nc.vector.tensor_copy(t_[m0:m0 + 32, :], ones_t[:])

            spst = {}  # (si) -> stats tiles, allocated at first set

            def prep_span(si, sets):
                """Stats + rstd + lhsT prep for span si, given qsets only
                (engines: PE stats, Act square/sqrt, DVE sub/recip/x*rr,
                Pool mu*rstd)."""
                s0, sw = SPANS[si]
                # A) per-band sums via PE: mu / meansq at quadrant rows
                if si not in spst:
                    spst[si] = (
                        {qs: stps.tile([128, SPAN], F32, tag="mu",
                                       name=f"mu{qs}", bufs=2) for qs in "AB"},
                        {qs: stps.tile([128, SPAN], F32, tag="ms",
                                       name=f"ms{qs}", bufs=1) for qs in "AB"},
                        # C set (P6): mu at row 0, meansq at row 32; shares
                        # the "ms" bank (each is consumed by its chain
                        # before the next set's stats land)
                        stps.tile([64, SPAN], F32, tag="ms", name="muC", bufs=1),
                        {qs: srp.tile([128, SPAN], F32, tag=f"sr{qs}",
                                      name=f"sr{qs}") for qs in QSETS},
                    )
                    spst.pop(si - 2, None)
                mu, msq, muc, srstd = spst[si]

                def mu_ap(p, q0, w=None):
                    w = p.n if w is None else w
                    if p.qset == "C":
                        return muc[0:w, :sw]
                    return mu[p.qset][q0:q0 + w, :sw]

                def msq_ap(p, q0, w=None):
                    w = p.n if w is None else w
                    if p.qset == "C":
                        return muc[32:32 + w, :sw]
                    return msq[p.qset][q0:q0 + w, :sw]

                # per-SET emission (stats -> chain -> pack prep) so each
                # in-order engine queue finishes set qs before touching
                # ops that depend on later-loaded packs.
                for qs in sets:
                    spacks = [p for p in PACKS if p.qset == qs]
                    nr = 1 if qs == "C" else 96  # rows in this set's chain
                    for p in spacks:
                        q0 = 32 * p.quad
                        xsq = xsqp.tile([128, SPAN], F32R, tag="xsq", name="xsq")
                        xin_f = xin(p.pid, s0, sw)
                        nc.gpsimd.tensor_tensor(
                            xsq[0:p.F, :sw], xin_f, xin_f, op=Alu.mult)
                        nc.tensor.matmul(
                            mu_ap(p, q0, 32), selx[p.pid].bitcast(F32),
                            xin_f.bitcast(F32), start=True, stop=True)
                        nc.tensor.matmul(
                            msq_ap(p, q0, 32), selx[p.pid].bitcast(F32),
                            xsq[0:p.F, :sw].bitcast(F32), start=True, stop=True)
                    # rstd = 1/sqrt(msq - mu^2 + eps), batched per set
                    mu_t = muc[0:1, :sw] if qs == "C" else mu[qs][0:nr, :sw]
                    ms_t = muc[32:33, :sw] if qs == "C" else msq[qs][0:nr, :sw]
                    musq = stt.tile([128, SPAN], F32, tag="musq", name="musq")
                    nc.scalar.activation(
                        musq[0:nr, :sw], mu_t, Act.Square)
                    var = stt.tile([128, SPAN], F32, tag="var", name="var")
                    nc.vector.tensor_tensor(
                        var[0:nr, :sw], ms_t,
                        musq[0:nr, :sw], op=Alu.subtract)
                    sq = musq  # reuse: musq's last reader (var) is done
                    nc.scalar.activation(
                        sq[0:nr, :sw], var[0:nr, :sw], Act.Sqrt,
                        bias=eps_t[0:nr, 0:1], scale=1.0)
                    nc.vector.reciprocal_approx_fast(
                        out=srstd[qs][0:nr, :sw], in_=sq[0:nr, :sw])
                    # pack lhsT prep: [x*rstd_rep ; mu*rstd ; ones]
                    for p in spacks:
                        q0 = 32 * p.quad
                        t_ = xpt[(p.pid, si % 2)]
                        nc.vector.tensor_tensor(
                            t_[p.F32:p.F32 + p.n, :sw], mu_ap(p, q0),
                            srstd[p.qset][q0:q0 + p.n, :sw], op=Alu.mult)
                        rr = rrps.tile([128, SPAN], F32, tag="rr", name="rr")
                        # plain fp32 matmul: srstd must stay fp32 for the
                        # reciprocal op, and PE has slack for 4 cyc/row here
                        nc.tensor.matmul(
                            rr[0:p.F, :sw],
                            erep[p.qset][q0:q0 + p.n, 0:p.F],
                            srstd[p.qset][q0:q0 + p.n, :sw],
                            start=True, stop=True)
                        nc.vector.tensor_tensor(
                            t_[0:p.F, :sw], xin(p.pid, s0, sw),
                            rr[0:p.F, :sw], op=Alu.mult)

            drain_i = 0
            prep_span(0, SET_ORDER)
            for si, (s0, sw) in enumerate(SPANS):
                for ci, c0 in enumerate(range(s0, s0 + sw, CHUNK)):
                    cw = min(CHUNK, s0 + sw - c0)
                    ob = obp.tile([128, OBCOLS], F32, tag="ob", name="ob")
                    for p in PACKS:
                        lhsT = xpt[(p.pid, si % 2)][0:p.K, c0 - s0:c0 - s0 + cw]
                        ob0 = p.k0 * C
                        for (n0, nw, kb0, nb) in _blocks(p):
                            op = outps.tile([128, 512], F32, tag="op", name="op")
                            nc.tensor.matmul(
                                op[0:cw, 0:nw], lhsT,
                                wext[p.pid][:, n0:n0 + nw],
                                start=True, stop=True)
                            eng = DRAIN_PAT[drain_i % len(DRAIN_PAT)]
                            if eng == "a":
                                nc.scalar.activation(
                                    ob[0:cw, ob0 + n0:ob0 + n0 + nw],
                                    op[0:cw, 0:nw], Act.Copy)
                            elif eng == "v":
                                nc.vector.tensor_copy(
                                    ob[0:cw, ob0 + n0:ob0 + n0 + nw],
                                    op[0:cw, 0:nw])
                            else:
                                nc.gpsimd.tensor_copy(
                                    ob[0:cw, ob0 + n0:ob0 + n0 + nw],
                                    op[0:cw, 0:nw])
                            drain_i += 1
                    dst = out_d[:, c0:c0 + cw, :]
                    dst = dst.rearrange("j t c -> t j c")
                    src = ob[0:cw, :].rearrange("t (j c) -> t j c", c=C)
                    nc.sync.dma_start(out=dst, in_=src)
                    # issue next span's prep mid-span in two slices so the
                    # in-order Act/DVE queues never absorb one big burst
                    if si + 1 < len(SPANS):
                        if ci == 1:
                            prep_span(si + 1, ["C", "A"])
                        elif ci == 2:
                            prep_span(si + 1, ["B"])

    nc.compile()
    _CACHE["nc"] = nc
    return nc


# ------------------------------------------------------------------ driver --
def kernel(**inputs):
    from concourse.bass_utils import run_bass_kernel_spmd

    x = np.ascontiguousarray(np.asarray(inputs["x"], np.float32))
    B = x.shape[0]
    assert x.shape == (8, 2, F_BINS, T)
    ext = _host_params(inputs)
    nc = _build()
    in_maps = []
    for b in range(B):
        m = {"x": x[b].reshape(2 * F_BINS, T)}
        m.update(ext)
        in_maps.append(m)
    res = run_bass_kernel_spmd(nc, in_maps, core_ids=list(range(8)))
    out = np.stack([res.results[b]["out"] for b in range(B)], axis=0)
    return out.astype(np.float32, copy=False)


# revision 44
# speedup vs baseline: 1.4966x; 1.0159x over previous
"""BandSplit (BSRNN) Trainium2 kernel.

Math per band k (31 bands over 257 freq bins, groups of band width 3/6/16/27):
  xg = x[b, :, band_bins, t] flattened to d = 2*bw features (torch order:
       bin-major, re/im minor)
  out[b, k, t, :] = LayerNorm_d(xg) @ W_k + b_k          (d -> C=128)

Algebraic refactor used here (per band, per t):
  mu    = mean_d(x),  var = mean_d(x^2) - mu^2,  rstd = rsqrt(var + eps)
  out   = rstd*(x @ Wg) - (mu*rstd)*(u) + bb
  with (host-precomputed)  Wg = gamma*W,  u = sum_d Wg,  bb = b + beta @ W.
So on device, for pack lhsT rows = [x*rstd_rep ; pad ; mu*rstd ; ones ; pad]
and rhs = [Wg blockdiag ; 0 ; -u blockdiag ; bb row ; 0], a single fp32r
matmul per (pack, t-chunk) emits the FINAL output tile (t x (n*128)) in PSUM.

All matmul operands are bitcast to float32r: same fp32 bytes, but the PE
streams 1 column/cycle (vs 4 for plain fp32) when the moving free dim >= 256.

Per 128-t chunk the 11 block PSUM tiles drain (Act/DVE/Pool rotation) into one
wide SBUF tile [128 x 31*128], shipped to DRAM by a single DMA (t-major, one
512B descriptor per (t, band) run) -- full 360 GB/s with ~1 HWDGE setup/chunk.

Pipeline: span s+1's stats/rstd/lhsT prep is issued between chunk 0 and
chunk 1 of span s, so per-engine program order never stalls the drain +
store stream at span boundaries. All input DMAs ride the SP queue in
consumption order (pack by pack, erep as soon as its packs are present);
output DMAs follow on the same queue.

Sharding: batch-parallel, core b handles x[b] (B=8 = n_cores).
"""

import numpy as np

T = 3000
C = 128
F_BINS = 257
EPS = 1e-5
GROUPS = [(10, 3), (12, 6), (8, 16), (1, 27)]  # (n_bands, bins_per_band)

SPAN = 512   # stats/prep span (free dim of PSUM bank)
CHUNK = 128  # output t-chunk (PSUM partition dim)
OBCOLS = 31 * C


# ---------------------------------------------------------------- metadata --
class Band:
    def __init__(self, g, i, f0, bw):
        self.g, self.i, self.f0, self.bw = g, i, f0, bw


BANDS = []
_f0 = 0
for _g, (_n, _bw) in enumerate(GROUPS):
    for _i in range(_n):
        BANDS.append(Band(_g, _i, _f0, _bw))
        _f0 += _bw
assert _f0 == F_BINS and len(BANDS) == 31


class Pack:
    def __init__(self, pid, band_ids, qset, quad):
        self.pid = pid
        self.band_ids = list(band_ids)
        self.n = len(self.band_ids)
        self.bw = BANDS[self.band_ids[0]].bw
        self.d = 2 * self.bw
        self.F = self.n * self.d                    # feature rows
        self.F32 = ((self.F + 31) // 32) * 32       # aligned start of mu'' rows
        self.K = self.F32 + 32                      # lhsT partition count
        self.row_ones = self.F32 + self.n
        self.k0 = self.band_ids[0]                  # first global band
        self.f0 = BANDS[self.k0].f0                 # first freq bin
        self.qset = qset                            # 'A' or 'B'
        self.quad = quad                            # quadrant row base /32
        assert self.K <= 128 and self.F32 + self.n + 1 <= self.K


# matmul tile_position bases are limited to {0, 32, 64} (quadrant-3 HW bug),
# so at most 3 packs share a stats/srstd tile set.
PACKS = [
    Pack(0, range(0, 10), 'A', 0),
    Pack(1, range(10, 16), 'A', 1),
    Pack(2, range(16, 22), 'A', 2),
    Pack(3, range(22, 25), 'B', 0),
    Pack(4, range(25, 28), 'B', 1),
    Pack(5, range(28, 30), 'B', 2),
    Pack(6, range(30, 31), 'C', 0),
]
QSETS = "ABC"
EREP_COLS = max(p.F for p in PACKS)  # 96
# emission / load order: C first (one tiny band -> its chain clears the
# in-order Act/DVE queues almost immediately), then A, then B.
SET_ORDER = ["C", "A", "B"]
LOAD_ORDER = [6, 0, 1, 2, 3, 4, 5]

SPANS = [(s0, min(SPAN, T - s0)) for s0 in range(0, T, SPAN)]

# drain engine per block, rotating: a=Act (fastest), v=DVE, p=Pool.
# 11 blocks/chunk with widths [512,512,256, 512,256, 512,256, 384, 384,
# 256, 128]; pattern balances per-chunk drain time across the three.
DRAIN_PAT = ["a", "v", "a", "a", "v", "a", "a", "a", "a", "v", "a"]


def _blocks(p):
    """512-wide column blocks of pack p's (n*128) output."""
    out = []
    for n0 in range(0, p.n * C, 512):
        nw = min(512, p.n * C - n0)
        out.append((n0, nw, p.k0 + n0 // C, nw // C))  # col0, width, band0, nbands
    return out


# ------------------------------------------------------------- host params --
def _host_params(inputs):
    f32 = np.float32
    ext = {}
    for p in PACKS:
        # selx rides as 32 extra columns of wext (rows 0:F) so one DMA
        # loads both; wext pad rows [F,F32) and [row_ones+1,K) stay home.
        wext = np.zeros((p.K, p.n * C + 32), f32)
        selx = wext[0:p.F, p.n * C:]  # view: full quadrant write
        F2 = p.F // 2
        for j, k in enumerate(p.band_ids):
            b = BANDS[k]
            W = np.asarray(inputs[f"g{b.g}_W"][b.i], f32)        # (d, C)
            gam = np.asarray(inputs[f"g{b.g}_gamma"][b.i], f32)  # (d,)
            bet = np.asarray(inputs[f"g{b.g}_beta"][b.i], f32)
            bias = np.asarray(inputs[f"g{b.g}_b"][b.i], f32)     # (C,)
            Wg = gam[:, None] * W
            cols = slice(j * C, (j + 1) * C)
            # device row layout is plane-major: row pl*F2 + j*bw + f
            # holds (plane pl, bin f) of band j == torch feature 2f+pl
            for pl in range(2):
                rows = slice(pl * F2 + j * p.bw, pl * F2 + (j + 1) * p.bw)
                wext[rows, cols] = Wg[2 * np.arange(p.bw) + pl]
                selx[rows, j] = 1.0 / p.d
            wext[p.F32 + j, cols] = -Wg.sum(0)
            wext[p.row_ones, cols] = bias + bet @ W
        ext[f"wext{p.pid}"] = wext
    for qs in QSETS:
        erep = np.zeros((128, EREP_COLS), f32)
        for p in PACKS:
            if p.qset != qs:
                continue
            F2 = p.F // 2
            for j in range(p.n):
                for pl in range(2):
                    r0 = pl * F2 + j * p.bw
                    erep[32 * p.quad + j, r0:r0 + p.bw] = 1.0
        ext[f"erep{qs}"] = erep
    return ext


# ------------------------------------------------------------ device build --
_CACHE = {}


def _build():
    if "nc" in _CACHE:
        return _CACHE["nc"]
    import concourse.bacc as bacc
    import concourse.tile as tile
    from concourse import mybir

    Alu = mybir.AluOpType
    Act = mybir.ActivationFunctionType
    F32 = mybir.dt.float32
    F32R = mybir.dt.float32r

    nc = bacc.Bacc("TRN2", target_bir_lowering=False, debug=False, num_devices=8)
    x_d = nc.dram_tensor("x", [2 * F_BINS, T], F32R, kind="ExternalInput")
    out_d = nc.dram_tensor("out", [31, T, C], F32, kind="ExternalOutput")
    wext_d = {p.pid: nc.dram_tensor(f"wext{p.pid}", [p.K, p.n * C + 32], F32R,
                                    kind="ExternalInput") for p in PACKS}
    erep_d = {qs: nc.dram_tensor(f"erep{qs}", [128, EREP_COLS], F32R,
                                 kind="ExternalInput") for qs in QSETS}

    with tile.TileContext(nc) as tc:
        with (
            tc.tile_pool(name="const", bufs=1) as const,
            tc.tile_pool(name="xsqp", bufs=2) as xsqp,
            tc.tile_pool(name="stt", bufs=2) as stt,
            tc.tile_pool(name="obp", bufs=2) as obp,
            tc.tile_pool(name="srp", bufs=1) as srp,
            tc.tile_pool(name="stps", bufs=1, space="PSUM") as stps,
            tc.tile_pool(name="rrps", bufs=1, space="PSUM") as rrps,
            tc.tile_pool(name="outps", bufs=4, space="PSUM") as outps,
        ):
            # ---- resident constants, loaded on the SP queue in the order
            # the compute pipeline consumes them (pack by pack; erep for a
            # qset as soon as all its packs are in flight).
            xin_t = {}
            wext = {}
            selx = {}
            erep = {}
            for p in PACKS:
                xin_t[p.pid] = const.tile([p.F, T], F32R, tag=f"xin{p.pid}", name=f"xin{p.pid}")
                wext[p.pid] = const.tile([p.K, p.n * C + 32], F32R, tag=f"wx{p.pid}", name=f"wx{p.pid}")
                selx[p.pid] = wext[p.pid][0:p.F, p.n * C:p.n * C + 32]
            for qs in QSETS:
                erep[qs] = const.tile([128, EREP_COLS], F32R, tag=f"er{qs}", name=f"er{qs}")

            def xin(pid, s0, sw):
                return xin_t[pid][:, s0:s0 + sw]

            for pid in LOAD_ORDER:
                p = PACKS[pid]
                F2 = p.F // 2
                # SBUF rows are plane-major: [all re bins | all im bins],
                # one contiguous 2D DMA per plane.
                for pl in range(2):
                    s_ = x_d[pl * F_BINS + p.f0:pl * F_BINS + p.f0 + F2, :]
                    d_ = xin_t[p.pid][pl * F2:(pl + 1) * F2, :]
                    nc.sync.dma_start(out=d_, in_=s_)
                # full K rows: the all-zero pad rows must be resident so
                # lhsT pad rows (set to 1.0) multiply against true zeros
                nc.sync.dma_start(out=wext[p.pid][:], in_=wext_d[p.pid][:])
                qs = p.qset
                if pid == max(q.pid for q in PACKS if q.qset == qs):
                    nc.sync.dma_start(out=erep[qs][:], in_=erep_d[qs][:])

            eps_t = const.tile([128, 1], F32, tag="epsc", name="epsc")
            nc.vector.memset(eps_t[:], EPS)

            # persistent double-buffered lhsT tiles (f32r). Rows
            # [floor32(F), K) are filled with 1.0 once: the ones row gets
            # its 1.0, pad rows become finite don't-cares (their wext rows
            # are zero), data rows in range are rewritten every span.
            # memset can't emit fp32r, so fill via fp32 scratch + DVE copy
            # (the copy rounds on write).
            ones_t = const.tile([32, SPAN], F32, tag="ones", name="ones")
            nc.gpsimd.memset(ones_t[:], 1.0)
            xpt = {}
            for p in PACKS:
                ms0 = (p.F // 32) * 32
                for par in range(2):
                    t_ = const.tile([p.K, SPAN], F32R, tag=f"xp{p.pid}_{par}",
                                    name=f"xp{p.pid}_{par}")
                    xpt[(p.pid, par)] = t_
                    for m0 in range(ms0, p.K, 32):
                        nc.vector.tensor_copy(t_[m0:m0 + 32, :], ones_t[:])

            spst = {}  # (si) -> stats tiles, allocated at first set

            def prep_span(si, sets):
                """Stats + rstd + lhsT prep for span si, given qsets only
                (engines: PE stats, Act square/sqrt, DVE sub/recip/x*rr,
                Pool mu*rstd)."""
                s0, sw = SPANS[si]
                # A) per-band sums via PE: mu / meansq at quadrant rows
                if si not in spst:
                    spst[si] = (
                        {qs: stps.tile([128, SPAN], F32, tag="mu",
                                       name=f"mu{qs}", bufs=2) for qs in "AB"},
                        {qs: stps.tile([128, SPAN], F32, tag="ms",
                                       name=f"ms{qs}", bufs=1) for qs in "AB"},
                        # C set (P6): mu at row 0, meansq at row 32; shares
                        # the "ms" bank (each is consumed by its chain
                        # before the next set's stats land)
                        stps.tile([64, SPAN], F32, tag="ms", name="muC", bufs=1),
                        {qs: srp.tile([128, SPAN], F32, tag=f"sr{qs}",
                                      name=f"sr{qs}") for qs in QSETS},
                        {qs: srp.tile([128, SPAN], F32R, tag=f"sq{qs}",
                                      name=f"sq{qs}") for qs in QSETS},
                    )
                    spst.pop(si - 2, None)
                mu, msq, muc, srstd, srstd_r = spst[si]

                def mu_ap(p, q0, w=None):
                    w = p.n if w is None else w
                    if p.qset == "C":
                        return muc[0:w, :sw]
                    return mu[p.qset][q0:q0 + w, :sw]

                def msq_ap(p, q0, w=None):
                    w = p.n if w is None else w
                    if p.qset == "C":
                        return muc[32:32 + w, :sw]
                    return msq[p.qset][q0:q0 + w, :sw]

                # per-SET emission (stats -> chain -> pack prep) so each
                # in-order engine queue finishes set qs before touching
                # ops that depend on later-loaded packs.
                for qs in sets:
                    spacks = [p for p in PACKS if p.qset == qs]
                    nr = 1 if qs == "C" else 96  # rows in this set's chain
                    for p in spacks:
                        q0 = 32 * p.quad
                        xsq = xsqp.tile([128, SPAN], F32R, tag="xsq", name="xsq")
                        xin_f = xin(p.pid, s0, sw)
                        nc.gpsimd.tensor_tensor(
                            xsq[0:p.F, :sw], xin_f, xin_f, op=Alu.mult)
                        nc.tensor.matmul(
                            mu_ap(p, q0, 32), selx[p.pid].bitcast(F32),
                            xin_f.bitcast(F32), start=True, stop=True)
                        nc.tensor.matmul(
                            msq_ap(p, q0, 32), selx[p.pid].bitcast(F32),
                            xsq[0:p.F, :sw].bitcast(F32), start=True, stop=True)
                    # rstd = 1/sqrt(msq - mu^2 + eps), batched per set
                    mu_t = muc[0:1, :sw] if qs == "C" else mu[qs][0:nr, :sw]
                    ms_t = muc[32:33, :sw] if qs == "C" else msq[qs][0:nr, :sw]
                    musq = stt.tile([128, SPAN], F32, tag="musq", name="musq")
                    nc.scalar.activation(
                        musq[0:nr, :sw], mu_t, Act.Square)
                    var = stt.tile([128, SPAN], F32, tag="var", name="var")
                    nc.vector.tensor_tensor(
                        var[0:nr, :sw], ms_t,
                        musq[0:nr, :sw], op=Alu.subtract)
                    sq = musq  # reuse: musq's last reader (var) is done
                    nc.scalar.activation(
                        sq[0:nr, :sw], var[0:nr, :sw], Act.Sqrt,
                        bias=eps_t[0:nr, 0:1], scale=1.0)
                    nc.vector.reciprocal_approx_fast(
                        out=srstd[qs][0:nr, :sw], in_=sq[0:nr, :sw])
                    r0q = 0 if qs == "C" else 0
                    nc.scalar.activation(
                        srstd_r[qs][0:nr, :sw], srstd[qs][0:nr, :sw],
                        Act.Copy)
                    # pack lhsT prep: [x*rstd_rep ; mu*rstd ; ones]
                    for p in spacks:
                        q0 = 32 * p.quad
                        t_ = xpt[(p.pid, si % 2)]
                        nc.vector.tensor_tensor(
                            t_[p.F32:p.F32 + p.n, :sw], mu_ap(p, q0),
                            srstd[p.qset][q0:q0 + p.n, :sw], op=Alu.mult)
                        rr = rrps.tile([128, SPAN], F32, tag="rr", name="rr")
                        nc.tensor.matmul(
                            rr[0:p.F, :sw],
                            erep[p.qset][q0:q0 + p.n, 0:p.F],
                            srstd_r[p.qset][q0:q0 + p.n, :sw],
                            start=True, stop=True)
                        nc.vector.tensor_tensor(
                            t_[0:p.F, :sw], xin(p.pid, s0, sw),
                            rr[0:p.F, :sw], op=Alu.mult)

            drain_i = 0
            prep_span(0, SET_ORDER)
            for si, (s0, sw) in enumerate(SPANS):
                for ci, c0 in enumerate(range(s0, s0 + sw, CHUNK)):
                    cw = min(CHUNK, s0 + sw - c0)
                    ob = obp.tile([128, OBCOLS], F32, tag="ob", name="ob")
                    for p in PACKS:
                        lhsT = xpt[(p.pid, si % 2)][0:p.K, c0 - s0:c0 - s0 + cw]
                        ob0 = p.k0 * C
                        for (n0, nw, kb0, nb) in _blocks(p):
                            op = outps.tile([128, 512], F32, tag="op", name="op")
                            nc.tensor.matmul(
                                op[0:cw, 0:nw], lhsT,
                                wext[p.pid][:, n0:n0 + nw],
                                start=True, stop=True)
                            eng = DRAIN_PAT[drain_i % len(DRAIN_PAT)]
                            if eng == "a":
                                nc.scalar.activation(
                                    ob[0:cw, ob0 + n0:ob0 + n0 + nw],
                                    op[0:cw, 0:nw], Act.Copy)
                            elif eng == "v":
                                nc.vector.tensor_copy(
                                    ob[0:cw, ob0 + n0:ob0 + n0 + nw],
                                    op[0:cw, 0:nw])
                            else:
                                nc.gpsimd.tensor_copy(
                                    ob[0:cw, ob0 + n0:ob0 + n0 + nw],
                                    op[0:cw, 0:nw])
                            drain_i += 1
                    dst = out_d[:, c0:c0 + cw, :]
                    dst = dst.rearrange("j t c -> t j c")
                    src = ob[0:cw, :].rearrange("t (j c) -> t j c", c=C)
                    nc.sync.dma_start(out=dst, in_=src)
                    # issue next span's prep mid-span in two slices so the
                    # in-order Act/DVE queues never absorb one big burst
                    if si + 1 < len(SPANS):
                        if ci == 1:
                            prep_span(si + 1, ["C", "A"])
                        elif ci == 2:
                            prep_span(si + 1, ["B"])

    nc.compile()
    _CACHE["nc"] = nc
    return nc


# ------------------------------------------------------------------ driver --
def kernel(**inputs):
    from concourse.bass_utils import run_bass_kernel_spmd

    x = np.ascontiguousarray(np.asarray(inputs["x"], np.float32))
    B = x.shape[0]
    assert x.shape == (8, 2, F_BINS, T)
    ext = _host_params(inputs)
    nc = _build()
    in_maps = []
    for b in range(B):
        m = {"x": x[b].reshape(2 * F_BINS, T)}
        m.update(ext)
        in_maps.append(m)
    res = run_bass_kernel_spmd(nc, in_maps, core_ids=list(range(8)))
    out = np.stack([res.results[b]["out"] for b in range(B)], axis=0)
    return out.astype(np.float32, copy=False)


# revision 46
# speedup vs baseline: 1.6059x; 1.0730x over previous
"""BandSplit (BSRNN) Trainium2 kernel.

Math per band k (31 bands over 257 freq bins, groups of band width 3/6/16/27):
  xg = x[b, :, band_bins, t] flattened to d = 2*bw features (torch order:
       bin-major, re/im minor)
  out[b, k, t, :] = LayerNorm_d(xg) @ W_k + b_k          (d -> C=128)

Algebraic refactor used here (per band, per t):
  mu    = mean_d(x),  var = mean_d(x^2) - mu^2,  rstd = rsqrt(var + eps)
  out   = rstd*(x @ Wg) - (mu*rstd)*(u) + bb
  with (host-precomputed)  Wg = gamma*W,  u = sum_d Wg,  bb = b + beta @ W.
So on device, for pack lhsT rows = [x*rstd_rep ; pad ; mu*rstd ; ones ; pad]
and rhs = [Wg blockdiag ; 0 ; -u blockdiag ; bb row ; 0], a single fp32r
matmul per (pack, t-chunk) emits the FINAL output tile (t x (n*128)) in PSUM.

All matmul operands are bitcast to float32r: same fp32 bytes, but the PE
streams 1 column/cycle (vs 4 for plain fp32) when the moving free dim >= 256.

Per 128-t chunk the 11 block PSUM tiles drain (Act/DVE/Pool rotation) into one
wide SBUF tile [128 x 31*128], shipped to DRAM by a single DMA (t-major, one
512B descriptor per (t, band) run) -- full 360 GB/s with ~1 HWDGE setup/chunk.

Pipeline: span s+1's stats/rstd/lhsT prep is issued between chunk 0 and
chunk 1 of span s, so per-engine program order never stalls the drain +
store stream at span boundaries. All input DMAs ride the SP queue in
consumption order (pack by pack, erep as soon as its packs are present);
output DMAs follow on the same queue.

Sharding: batch-parallel, core b handles x[b] (B=8 = n_cores).
"""

import numpy as np

T = 3000
C = 128
F_BINS = 257
EPS = 1e-5
GROUPS = [(10, 3), (12, 6), (8, 16), (1, 27)]  # (n_bands, bins_per_band)

SPAN = 512   # stats/prep span (free dim of PSUM bank)
CHUNK = 128  # output t-chunk (PSUM partition dim)
OBCOLS = 31 * C


# ---------------------------------------------------------------- metadata --
class Band:
    def __init__(self, g, i, f0, bw):
        self.g, self.i, self.f0, self.bw = g, i, f0, bw


BANDS = []
_f0 = 0
for _g, (_n, _bw) in enumerate(GROUPS):
    for _i in range(_n):
        BANDS.append(Band(_g, _i, _f0, _bw))
        _f0 += _bw
assert _f0 == F_BINS and len(BANDS) == 31


class Pack:
    def __init__(self, pid, band_ids, qset, quad):
        self.pid = pid
        self.band_ids = list(band_ids)
        self.n = len(self.band_ids)
        self.bw = BANDS[self.band_ids[0]].bw
        self.d = 2 * self.bw
        self.F = self.n * self.d                    # feature rows
        self.F32 = ((self.F + 31) // 32) * 32       # aligned start of mu'' rows
        self.K = self.F32 + 32                      # lhsT partition count
        self.row_ones = self.F32 + self.n
        self.k0 = self.band_ids[0]                  # first global band
        self.f0 = BANDS[self.k0].f0                 # first freq bin
        self.qset = qset                            # 'A' or 'B'
        self.quad = quad                            # quadrant row base /32
        assert self.K <= 128 and self.F32 + self.n + 1 <= self.K


# matmul tile_position bases are limited to {0, 32, 64} (quadrant-3 HW bug),
# so at most 3 packs share a stats/srstd tile set.
PACKS = [
    Pack(0, range(0, 10), 'A', 0),
    Pack(1, range(10, 16), 'A', 1),
    Pack(2, range(16, 22), 'A', 2),
    Pack(3, range(22, 25), 'B', 0),
    Pack(4, range(25, 28), 'B', 1),
    Pack(5, range(28, 30), 'B', 2),
    Pack(6, range(30, 31), 'C', 0),
]
QSETS = "ABC"
EREP_COLS = max(p.F for p in PACKS)  # 96
# emission / load order: C first (one tiny band -> its chain clears the
# in-order Act/DVE queues almost immediately), then A, then B.
SET_ORDER = ["C", "A", "B"]
LOAD_ORDER = [6, 0, 1, 2, 3, 4, 5]

SPANS = [(s0, min(SPAN, T - s0)) for s0 in range(0, T, SPAN)]

# drain engine per block, rotating: a=Act (fastest), v=DVE, p=Pool.
# 11 blocks/chunk with widths [512,512,256, 512,256, 512,256, 384, 384,
# 256, 128]; pattern balances per-chunk drain time across the three.
DRAIN_PAT = ["a", "v", "a", "a", "v", "a", "a", "a", "a", "v", "a"]


def _blocks(p):
    """512-wide column blocks of pack p's (n*128) output."""
    out = []
    for n0 in range(0, p.n * C, 512):
        nw = min(512, p.n * C - n0)
        out.append((n0, nw, p.k0 + n0 // C, nw // C))  # col0, width, band0, nbands
    return out


# ------------------------------------------------------------- host params --
def _host_params(inputs):
    f32 = np.float32
    ext = {}
    for p in PACKS:
        # selx rides as 128 extra columns of wext (rows 0:F) so one DMA
        # loads both. The band quadrant offset lives in selx's COLUMNS
        # (32*quad + j) so the f32r stats matmuls write at output
        # partition base 0 (fp32r + nonzero tile_position col base is
        # rejected by codegen); a set's packs accumulate into one tile.
        wext = np.zeros((p.K, p.n * C + 128), f32)
        selx = wext[0:p.F, p.n * C:]
        F2 = p.F // 2
        for j, k in enumerate(p.band_ids):
            b = BANDS[k]
            W = np.asarray(inputs[f"g{b.g}_W"][b.i], f32)        # (d, C)
            gam = np.asarray(inputs[f"g{b.g}_gamma"][b.i], f32)  # (d,)
            bet = np.asarray(inputs[f"g{b.g}_beta"][b.i], f32)
            bias = np.asarray(inputs[f"g{b.g}_b"][b.i], f32)     # (C,)
            Wg = gam[:, None] * W
            cols = slice(j * C, (j + 1) * C)
            # device row layout is plane-major: row pl*F2 + j*bw + f
            # holds (plane pl, bin f) of band j == torch feature 2f+pl
            for pl in range(2):
                rows = slice(pl * F2 + j * p.bw, pl * F2 + (j + 1) * p.bw)
                wext[rows, cols] = Wg[2 * np.arange(p.bw) + pl]
                selx[rows, 32 * p.quad + j] = 1.0 / p.d
            wext[p.F32 + j, cols] = -Wg.sum(0)
            wext[p.row_ones, cols] = bias + bet @ W
        ext[f"wext{p.pid}"] = wext
    for qs in QSETS:
        erep = np.zeros((128, EREP_COLS), f32)
        for p in PACKS:
            if p.qset != qs:
                continue
            F2 = p.F // 2
            for j in range(p.n):
                for pl in range(2):
                    r0 = pl * F2 + j * p.bw
                    erep[32 * p.quad + j, r0:r0 + p.bw] = 1.0
        ext[f"erep{qs}"] = erep
    return ext


# ------------------------------------------------------------ device build --
_CACHE = {}


def _build():
    if "nc" in _CACHE:
        return _CACHE["nc"]
    import concourse.bacc as bacc
    import concourse.tile as tile
    from concourse import mybir

    Alu = mybir.AluOpType
    Act = mybir.ActivationFunctionType
    F32 = mybir.dt.float32
    F32R = mybir.dt.float32r

    nc = bacc.Bacc("TRN2", target_bir_lowering=False, debug=False, num_devices=8)
    x_d = nc.dram_tensor("x", [2 * F_BINS, T], F32R, kind="ExternalInput")
    out_d = nc.dram_tensor("out", [31, T, C], F32, kind="ExternalOutput")
    wext_d = {p.pid: nc.dram_tensor(f"wext{p.pid}", [p.K, p.n * C + 128], F32R,
                                    kind="ExternalInput") for p in PACKS}
    erep_d = {qs: nc.dram_tensor(f"erep{qs}", [128, EREP_COLS], F32R,
                                 kind="ExternalInput") for qs in QSETS}

    with tile.TileContext(nc) as tc:
        with (
            tc.tile_pool(name="const", bufs=1) as const,
            tc.tile_pool(name="xsqp", bufs=2) as xsqp,
            tc.tile_pool(name="stt", bufs=2) as stt,
            tc.tile_pool(name="obp", bufs=2) as obp,
            tc.tile_pool(name="srp", bufs=1) as srp,
            tc.tile_pool(name="stps", bufs=1, space="PSUM") as stps,
            tc.tile_pool(name="rrps", bufs=1, space="PSUM") as rrps,
            tc.tile_pool(name="outps", bufs=3, space="PSUM") as outps,
        ):
            # ---- resident constants, loaded on the SP queue in the order
            # the compute pipeline consumes them (pack by pack; erep for a
            # qset as soon as all its packs are in flight).
            xin_t = {}
            wext = {}
            selx = {}
            erep = {}
            for p in PACKS:
                xin_t[p.pid] = const.tile([p.F, T], F32R, tag=f"xin{p.pid}", name=f"xin{p.pid}")
                wext[p.pid] = const.tile([p.K, p.n * C + 128], F32R, tag=f"wx{p.pid}", name=f"wx{p.pid}")
                selx[p.pid] = wext[p.pid][0:p.F, p.n * C:p.n * C + 128]
            for qs in QSETS:
                erep[qs] = const.tile([128, EREP_COLS], F32R, tag=f"er{qs}", name=f"er{qs}")

            def xin(pid, s0, sw):
                return xin_t[pid][:, s0:s0 + sw]

            for pid in LOAD_ORDER:
                p = PACKS[pid]
                F2 = p.F // 2
                # SBUF rows are plane-major: [all re bins | all im bins],
                # one contiguous 2D DMA per plane.
                for pl in range(2):
                    s_ = x_d[pl * F_BINS + p.f0:pl * F_BINS + p.f0 + F2, :]
                    d_ = xin_t[p.pid][pl * F2:(pl + 1) * F2, :]
                    nc.sync.dma_start(out=d_, in_=s_)
                # full K rows: the all-zero pad rows must be resident so
                # lhsT pad rows (set to 1.0) multiply against true zeros
                nc.sync.dma_start(out=wext[p.pid][:], in_=wext_d[p.pid][:])
                qs = p.qset
                if pid == max(q.pid for q in PACKS if q.qset == qs):
                    nc.sync.dma_start(out=erep[qs][:], in_=erep_d[qs][:])

            eps_t = const.tile([128, 1], F32, tag="epsc", name="epsc")
            nc.vector.memset(eps_t[:], EPS)

            # persistent double-buffered lhsT tiles (f32r). Rows
            # [floor32(F), K) are filled with 1.0 once: the ones row gets
            # its 1.0, pad rows become finite don't-cares (their wext rows
            # are zero), data rows in range are rewritten every span.
            # memset can't emit fp32r, so fill via fp32 scratch + DVE copy
            # (the copy rounds on write).
            ones_t = const.tile([32, SPAN], F32, tag="ones", name="ones")
            nc.gpsimd.memset(ones_t[:], 1.0)
            xpt = {}
            for p in PACKS:
                ms0 = (p.F // 32) * 32
                for par in range(2):
                    t_ = const.tile([p.K, SPAN], F32R, tag=f"xp{p.pid}_{par}",
                                    name=f"xp{p.pid}_{par}")
                    xpt[(p.pid, par)] = t_
                    for m0 in range(ms0, p.K, 32):
                        nc.vector.tensor_copy(t_[m0:m0 + 32, :], ones_t[:])

            spst = {}  # (si) -> stats tiles, allocated at first set

            def prep_span(si, sets):
                """Stats + rstd + lhsT prep for span si, given qsets only
                (engines: PE stats, Act square/sqrt, DVE sub/recip/x*rr,
                Pool mu*rstd)."""
                s0, sw = SPANS[si]
                # A) per-band sums via PE: mu / meansq at quadrant rows
                if si not in spst:
                    spst[si] = (
                        {qs: stps.tile([128, SPAN], F32, tag="mu",
                                       name=f"mu{qs}", bufs=2) for qs in "ABC"},
                        {qs: stps.tile([128, SPAN], F32, tag="ms",
                                       name=f"ms{qs}", bufs=2) for qs in "ABC"},
                        None,
                        {qs: srp.tile([128, SPAN], F32, tag=f"sr{qs}",
                                      name=f"sr{qs}") for qs in QSETS},
                        {qs: srp.tile([128, SPAN], F32R, tag=f"sq{qs}",
                                      name=f"sq{qs}") for qs in QSETS},
                    )
                    spst.pop(si - 2, None)
                mu, msq, _, srstd, srstd_r = spst[si]

                def mu_ap(p, q0, w=None):
                    w = p.n if w is None else w
                    return mu[p.qset][q0:q0 + w, :sw]

                def msq_ap(p, q0, w=None):
                    w = p.n if w is None else w
                    return msq[p.qset][q0:q0 + w, :sw]

                # per-SET emission (stats -> chain -> pack prep) so each
                # in-order engine queue finishes set qs before touching
                # ops that depend on later-loaded packs.
                for qs in sets:
                    spacks = [p for p in PACKS if p.qset == qs]
                    nr = 1 if qs == "C" else 96  # rows in this set's chain
                    xsqs = {}
                    for p in spacks:
                        xsq = xsqp.tile([128, SPAN], F32R, tag="xsq", name="xsq")
                        xsqs[p.pid] = xsq
                        xin_f = xin(p.pid, s0, sw)
                        nc.gpsimd.tensor_tensor(
                            xsq[0:p.F, :sw], xin_f, xin_f, op=Alu.mult)
                    # per-band sums: the set's packs ACCUMULATE into one
                    # [128, sw] tile (each selx writes only its quadrant
                    # rows via its column offsets; the rest add zeros)
                    np_ = len(spacks)
                    for pi, p in enumerate(spacks):
                        nc.tensor.matmul(
                            mu[qs][:, :sw], selx[p.pid],
                            xin(p.pid, s0, sw),
                            start=(pi == 0), stop=(pi == np_ - 1))
                    for pi, p in enumerate(spacks):
                        nc.tensor.matmul(
                            msq[qs][:, :sw], selx[p.pid],
                            xsqs[p.pid][0:p.F, :sw],
                            start=(pi == 0), stop=(pi == np_ - 1))
                    # rstd = 1/sqrt(msq - mu^2 + eps), batched per set
                    mu_t = mu[qs][0:nr, :sw]
                    ms_t = msq[qs][0:nr, :sw]
                    musq = stt.tile([128, SPAN], F32, tag="musq", name="musq")
                    nc.scalar.activation(
                        musq[0:nr, :sw], mu_t, Act.Square)
                    var = stt.tile([128, SPAN], F32, tag="var", name="var")
                    nc.vector.tensor_tensor(
                        var[0:nr, :sw], ms_t,
                        musq[0:nr, :sw], op=Alu.subtract)
                    sq = musq  # reuse: musq's last reader (var) is done
                    nc.scalar.activation(
                        sq[0:nr, :sw], var[0:nr, :sw], Act.Sqrt,
                        bias=eps_t[0:nr, 0:1], scale=1.0)
                    nc.vector.reciprocal_approx_fast(
                        out=srstd[qs][0:nr, :sw], in_=sq[0:nr, :sw])
                    nc.scalar.activation(
                        srstd_r[qs][0:nr, :sw], srstd[qs][0:nr, :sw],
                        Act.Copy)
                    # pack lhsT prep: [x*rstd_rep ; mu*rstd ; ones]
                    for p in spacks:
                        q0 = 32 * p.quad
                        t_ = xpt[(p.pid, si % 2)]
                        nc.vector.tensor_tensor(
                            t_[p.F32:p.F32 + p.n, :sw], mu_ap(p, q0),
                            srstd[p.qset][q0:q0 + p.n, :sw], op=Alu.mult)
                        rr = rrps.tile([128, SPAN], F32, tag="rr", name="rr")
                        nc.tensor.matmul(
                            rr[0:p.F, :sw],
                            erep[p.qset][q0:q0 + p.n, 0:p.F],
                            srstd_r[p.qset][q0:q0 + p.n, :sw],
                            start=True, stop=True)
                        nc.vector.tensor_tensor(
                            t_[0:p.F, :sw], xin(p.pid, s0, sw),
                            rr[0:p.F, :sw], op=Alu.mult)

            drain_i = 0
            prep_span(0, SET_ORDER)
            for si, (s0, sw) in enumerate(SPANS):
                for ci, c0 in enumerate(range(s0, s0 + sw, CHUNK)):
                    cw = min(CHUNK, s0 + sw - c0)
                    ob = obp.tile([128, OBCOLS], F32, tag="ob", name="ob")
                    for p in PACKS:
                        lhsT = xpt[(p.pid, si % 2)][0:p.K, c0 - s0:c0 - s0 + cw]
                        ob0 = p.k0 * C
                        for (n0, nw, kb0, nb) in _blocks(p):
                            op = outps.tile([128, 512], F32, tag="op", name="op")
                            nc.tensor.matmul(
                                op[0:cw, 0:nw], lhsT,
                                wext[p.pid][:, n0:n0 + nw],
                                start=True, stop=True)
                            eng = DRAIN_PAT[drain_i % len(DRAIN_PAT)]
                            if eng == "a":
                                nc.scalar.activation(
                                    ob[0:cw, ob0 + n0:ob0 + n0 + nw],
                                    op[0:cw, 0:nw], Act.Copy)
                            elif eng == "v":
                                nc.vector.tensor_copy(
                                    ob[0:cw, ob0 + n0:ob0 + n0 + nw],
                                    op[0:cw, 0:nw])
                            else:
                                nc.gpsimd.tensor_copy(
                                    ob[0:cw, ob0 + n0:ob0 + n0 + nw],
                                    op[0:cw, 0:nw])
                            drain_i += 1
                    dst = out_d[:, c0:c0 + cw, :]
                    dst = dst.rearrange("j t c -> t j c")
                    src = ob[0:cw, :].rearrange("t (j c) -> t j c", c=C)
                    nc.sync.dma_start(out=dst, in_=src)
                    # issue next span's prep mid-span in two slices so the
                    # in-order Act/DVE queues never absorb one big burst
                    if si + 1 < len(SPANS):
                        if ci == 1:
                            prep_span(si + 1, ["C", "A"])
                        elif ci == 2:
                            prep_span(si + 1, ["B"])

    nc.compile()
    _CACHE["nc"] = nc
    return nc


# ------------------------------------------------------------------ driver --
def kernel(**inputs):
    from concourse.bass_utils import run_bass_kernel_spmd

    x = np.ascontiguousarray(np.asarray(inputs["x"], np.float32))
    B = x.shape[0]
    assert x.shape == (8, 2, F_BINS, T)
    ext = _host_params(inputs)
    nc = _build()
    in_maps = []
    for b in range(B):
        m = {"x": x[b].reshape(2 * F_BINS, T)}
        m.update(ext)
        in_maps.append(m)
    res = run_bass_kernel_spmd(nc, in_maps, core_ids=list(range(8)))
    out = np.stack([res.results[b]["out"] for b in range(B)], axis=0)
    return out.astype(np.float32, copy=False)


# revision 56
# speedup vs baseline: 1.6600x; 1.0337x over previous
"""BandSplit (BSRNN) Trainium2 kernel.

Math per band k (31 bands over 257 freq bins, groups of band width 3/6/16/27):
  xg = x[b, :, band_bins, t] flattened to d = 2*bw features (torch order:
       bin-major, re/im minor)
  out[b, k, t, :] = LayerNorm_d(xg) @ W_k + b_k          (d -> C=128)

Algebraic refactor used here (per band, per t):
  mu    = mean_d(x),  var = mean_d(x^2) - mu^2,  rstd = rsqrt(var + eps)
  out   = rstd*(x @ Wg) - (mu*rstd)*(u) + bb
  with (host-precomputed)  Wg = gamma*W,  u = sum_d Wg,  bb = b + beta @ W.
So on device, for pack lhsT rows = [x*rstd_rep ; pad ; mu*rstd ; ones ; pad]
and rhs = [Wg blockdiag ; 0 ; -u blockdiag ; bb row ; 0], a single fp32r
matmul per (pack, t-chunk) emits the FINAL output tile (t x (n*128)) in PSUM.

All matmul inputs are float32r (TF32-like rounded fp32): the PE streams
1 column/cycle instead of 4 when the moving free dim >= 256, a 4x speedup
worth ~4e-3 relative error (gate is 2e-2). fp32r matmuls require output
partition base 0, so the per-band stats sums put the band quadrant offset
in selx's COLUMNS and each qset's packs accumulate into one PSUM tile.

Per 128-t chunk the 11 block PSUM tiles drain (Act/DVE rotation -- GPSIMD
cannot touch PSUM) into one wide SBUF tile [128 x 31*128], shipped to DRAM
by a single DMA (t-major, one 512B descriptor per (t, band) run) -- full
360 GB/s with ~1 HWDGE setup per chunk.

Pipeline: span s+1's stats/rstd/lhsT prep is issued one qset per chunk
slot of span s, so the in-order Act/DVE queues absorb small prep bursts
between drain work instead of stalling the store stream at span
boundaries. All DMAs ride the SP queue in consumption order.

Sharding: batch-parallel, core b handles x[b] (B=8 = n_cores).
"""

import numpy as np

T = 3000
C = 128
F_BINS = 257
EPS = 1e-5
GROUPS = [(10, 3), (12, 6), (8, 16), (1, 27)]  # (n_bands, bins_per_band)

SPAN = 512   # stats/prep span (free dim of PSUM bank)
CHUNK = 128  # output t-chunk (PSUM partition dim)
OBCOLS = 31 * C


# ---------------------------------------------------------------- metadata --
class Band:
    def __init__(self, g, i, f0, bw):
        self.g, self.i, self.f0, self.bw = g, i, f0, bw


BANDS = []
_f0 = 0
for _g, (_n, _bw) in enumerate(GROUPS):
    for _i in range(_n):
        BANDS.append(Band(_g, _i, _f0, _bw))
        _f0 += _bw
assert _f0 == F_BINS and len(BANDS) == 31


class Pack:
    def __init__(self, pid, band_ids, qset, quad):
        self.pid = pid
        self.band_ids = list(band_ids)
        self.n = len(self.band_ids)
        self.bw = BANDS[self.band_ids[0]].bw
        self.d = 2 * self.bw
        self.F = self.n * self.d                    # feature rows
        self.F32 = ((self.F + 31) // 32) * 32       # aligned start of mu'' rows
        self.K = self.F32 + 32                      # lhsT partition count
        self.row_ones = self.F32 + self.n
        self.k0 = self.band_ids[0]                  # first global band
        self.f0 = BANDS[self.k0].f0                 # first freq bin
        self.qset = qset                            # 'A' or 'B'
        self.quad = quad                            # quadrant row base /32
        assert self.K <= 128 and self.F32 + self.n + 1 <= self.K


# matmul tile_position bases are limited to {0, 32, 64} (quadrant-3 HW bug),
# so at most 3 packs share a stats/srstd tile set.
PACKS = [
    Pack(0, range(0, 10), 'A', 0),
    Pack(1, range(10, 16), 'A', 1),
    Pack(2, range(16, 22), 'A', 2),
    Pack(3, range(22, 25), 'B', 0),
    Pack(4, range(25, 28), 'B', 1),
    Pack(5, range(28, 30), 'B', 2),
    Pack(6, range(30, 31), 'C', 0),
]
QSETS = "ABC"
EREP_COLS = max(p.F for p in PACKS)  # 96
# emission / load order: C first (one tiny band -> its chain clears the
# in-order Act/DVE queues almost immediately), then A, then B.
SET_ORDER = ["C", "A", "B"]
LOAD_ORDER = [6, 0, 1, 2, 3, 4, 5]

SPANS = [(s0, min(SPAN, T - s0)) for s0 in range(0, T, SPAN)]

# drain engine per block: a=Act (0.83ns/col), v=DVE (1.04ns/col); Pool
# cannot read PSUM. 11 blocks/chunk, widths [512,512,256, 512,256,
# 512,256, 384, 384, 256, 128]; split tuned against the timeline sim.
DRAIN_PAT = ["a", "v", "v", "a", "v", "a", "v", "a", "a", "v", "a"]


def _blocks(p):
    """512-wide column blocks of pack p's (n*128) output."""
    out = []
    for n0 in range(0, p.n * C, 512):
        nw = min(512, p.n * C - n0)
        out.append((n0, nw, p.k0 + n0 // C, nw // C))  # col0, width, band0, nbands
    return out


# ------------------------------------------------------------- host params --
def _host_params(inputs):
    f32 = np.float32
    ext = {}
    for p in PACKS:
        # selx rides as 128 extra columns of wext (rows 0:F) so one DMA
        # loads both. The band quadrant offset lives in selx's COLUMNS
        # (32*quad + j) so the f32r stats matmuls write at output
        # partition base 0 (fp32r + nonzero tile_position col base is
        # rejected by codegen); a set's packs accumulate into one tile.
        wext = np.zeros((p.K, p.n * C + 128), f32)
        selx = wext[0:p.F, p.n * C:]
        F2 = p.F // 2
        for j, k in enumerate(p.band_ids):
            b = BANDS[k]
            W = np.asarray(inputs[f"g{b.g}_W"][b.i], f32)        # (d, C)
            gam = np.asarray(inputs[f"g{b.g}_gamma"][b.i], f32)  # (d,)
            bet = np.asarray(inputs[f"g{b.g}_beta"][b.i], f32)
            bias = np.asarray(inputs[f"g{b.g}_b"][b.i], f32)     # (C,)
            Wg = gam[:, None] * W
            cols = slice(j * C, (j + 1) * C)
            # device row layout is plane-major: row pl*F2 + j*bw + f
            # holds (plane pl, bin f) of band j == torch feature 2f+pl
            for pl in range(2):
                rows = slice(pl * F2 + j * p.bw, pl * F2 + (j + 1) * p.bw)
                wext[rows, cols] = Wg[2 * np.arange(p.bw) + pl]
                selx[rows, 32 * p.quad + j] = 1.0 / p.d
            wext[p.F32 + j, cols] = -Wg.sum(0)
            wext[p.row_ones, cols] = bias + bet @ W
        ext[f"wext{p.pid}"] = wext
    for qs in QSETS:
        erep = np.zeros((128, EREP_COLS), f32)
        for p in PACKS:
            if p.qset != qs:
                continue
            F2 = p.F // 2
            for j in range(p.n):
                for pl in range(2):
                    r0 = pl * F2 + j * p.bw
                    erep[32 * p.quad + j, r0:r0 + p.bw] = 1.0
        ext[f"erep{qs}"] = erep
    return ext


# ------------------------------------------------------------ device build --
_CACHE = {}


def _build():
    if "nc" in _CACHE:
        return _CACHE["nc"]
    import concourse.bacc as bacc
    import concourse.tile as tile
    from concourse import mybir

    Alu = mybir.AluOpType
    Act = mybir.ActivationFunctionType
    F32 = mybir.dt.float32
    F32R = mybir.dt.float32r

    nc = bacc.Bacc("TRN2", target_bir_lowering=False, debug=False, num_devices=8)
    x_d = nc.dram_tensor("x", [2 * F_BINS, T], F32R, kind="ExternalInput")
    out_d = nc.dram_tensor("out", [31, T, C], F32, kind="ExternalOutput")
    wext_d = {p.pid: nc.dram_tensor(f"wext{p.pid}", [p.K, p.n * C + 128], F32R,
                                    kind="ExternalInput") for p in PACKS}
    erep_d = {qs: nc.dram_tensor(f"erep{qs}", [128, EREP_COLS], F32R,
                                 kind="ExternalInput") for qs in QSETS}

    with tile.TileContext(nc) as tc:
        with (
            tc.tile_pool(name="const", bufs=1) as const,
            tc.tile_pool(name="xsqp", bufs=2) as xsqp,
            tc.tile_pool(name="stt", bufs=2) as stt,
            tc.tile_pool(name="obp", bufs=2) as obp,
            tc.tile_pool(name="srp", bufs=1) as srp,
            tc.tile_pool(name="stps", bufs=1, space="PSUM") as stps,
            tc.tile_pool(name="rrps", bufs=1, space="PSUM") as rrps,
            tc.tile_pool(name="outps", bufs=4, space="PSUM") as outps,
        ):
            # ---- resident constants, loaded on the SP queue in the order
            # the compute pipeline consumes them (pack by pack; erep for a
            # qset as soon as all its packs are in flight).
            xin_t = {}
            wext = {}
            selx = {}
            erep = {}
            for p in PACKS:
                xin_t[p.pid] = const.tile([p.F, T], F32R, tag=f"xin{p.pid}", name=f"xin{p.pid}")
                wext[p.pid] = const.tile([p.K, p.n * C + 128], F32R, tag=f"wx{p.pid}", name=f"wx{p.pid}")
                selx[p.pid] = wext[p.pid][0:p.F, p.n * C:p.n * C + 128]
            for qs in QSETS:
                erep[qs] = const.tile([128, EREP_COLS], F32R, tag=f"er{qs}", name=f"er{qs}")

            def xin(pid, s0, sw):
                return xin_t[pid][:, s0:s0 + sw]

            for pid in LOAD_ORDER:
                p = PACKS[pid]
                F2 = p.F // 2
                # SBUF rows are plane-major: [all re bins | all im bins],
                # one contiguous 2D DMA per plane.
                for pl in range(2):
                    s_ = x_d[pl * F_BINS + p.f0:pl * F_BINS + p.f0 + F2, :]
                    d_ = xin_t[p.pid][pl * F2:(pl + 1) * F2, :]
                    nc.sync.dma_start(out=d_, in_=s_)
                # full K rows: the all-zero pad rows must be resident so
                # lhsT pad rows (set to 1.0) multiply against true zeros
                nc.sync.dma_start(out=wext[p.pid][:], in_=wext_d[p.pid][:])
                qs = p.qset
                if pid == max(q.pid for q in PACKS if q.qset == qs):
                    nc.sync.dma_start(out=erep[qs][:], in_=erep_d[qs][:])

            eps_t = const.tile([128, 1], F32, tag="epsc", name="epsc")
            nc.vector.memset(eps_t[:], EPS)

            # persistent double-buffered lhsT tiles (f32r). Rows
            # [floor32(F), K) are filled with 1.0 once: the ones row gets
            # its 1.0, pad rows become finite don't-cares (their wext rows
            # are zero), data rows in range are rewritten every span.
            # memset can't emit fp32r, so fill via fp32 scratch + DVE copy
            # (the copy rounds on write).
            ones_t = const.tile([32, SPAN], F32, tag="ones", name="ones")
            nc.gpsimd.memset(ones_t[:], 1.0)
            xpt = {}
            for p in PACKS:
                ms0 = (p.F // 32) * 32
                for par in range(2):
                    t_ = const.tile([p.K, SPAN], F32R, tag=f"xp{p.pid}_{par}",
                                    name=f"xp{p.pid}_{par}")
                    xpt[(p.pid, par)] = t_
                    for m0 in range(ms0, p.K, 32):
                        nc.vector.tensor_copy(t_[m0:m0 + 32, :], ones_t[:])

            spst = {}  # (si) -> stats tiles, allocated at first set

            def prep_span(si, sets):
                """Stats + rstd + lhsT prep for span si, given qsets only
                (engines: PE stats, Act square/sqrt, DVE sub/recip/x*rr,
                Pool mu*rstd)."""
                s0, sw = SPANS[si]
                # A) per-band sums via PE: mu / meansq at quadrant rows
                if si not in spst:
                    spst[si] = (
                        {qs: stps.tile([128, SPAN], F32, tag="mu",
                                       name=f"mu{qs}", bufs=2) for qs in "ABC"},
                        {qs: stps.tile([128, SPAN], F32, tag="ms",
                                       name=f"ms{qs}", bufs=1) for qs in "ABC"},
                        None,
                        {qs: srp.tile([128, SPAN], F32, tag=f"sr{qs}",
                                      name=f"sr{qs}") for qs in QSETS},
                        {qs: srp.tile([128, SPAN], F32R, tag=f"sq{qs}",
                                      name=f"sq{qs}") for qs in QSETS},
                    )
                    spst.pop(si - 2, None)
                mu, msq, _, srstd, srstd_r = spst[si]

                def mu_ap(p, q0, w=None):
                    w = p.n if w is None else w
                    return mu[p.qset][q0:q0 + w, :sw]

                def msq_ap(p, q0, w=None):
                    w = p.n if w is None else w
                    return msq[p.qset][q0:q0 + w, :sw]

                # per-SET emission (stats -> chain -> pack prep) so each
                # in-order engine queue finishes set qs before touching
                # ops that depend on later-loaded packs.
                for qs in sets:
                    spacks = [p for p in PACKS if p.qset == qs]
                    nr = 1 if qs == "C" else 96  # rows in this set's chain
                    xsqs = {}
                    for p in spacks:
                        xsq = xsqp.tile([128, SPAN], F32R, tag="xsq", name="xsq")
                        xsqs[p.pid] = xsq
                        xin_f = xin(p.pid, s0, sw)
                        nc.gpsimd.tensor_tensor(
                            xsq[0:p.F, :sw], xin_f, xin_f, op=Alu.mult)
                    # per-band sums: the set's packs ACCUMULATE into one
                    # [128, sw] tile (each selx writes only its quadrant
                    # rows via its column offsets; the rest add zeros)
                    np_ = len(spacks)
                    for pi, p in enumerate(spacks):
                        nc.tensor.matmul(
                            mu[qs][:, :sw], selx[p.pid],
                            xin(p.pid, s0, sw),
                            start=(pi == 0), stop=(pi == np_ - 1))
                    for pi, p in enumerate(spacks):
                        nc.tensor.matmul(
                            msq[qs][:, :sw], selx[p.pid],
                            xsqs[p.pid][0:p.F, :sw],
                            start=(pi == 0), stop=(pi == np_ - 1))
                    # rstd = 1/sqrt(msq - mu^2 + eps), batched per set
                    mu_t = mu[qs][0:nr, :sw]
                    ms_t = msq[qs][0:nr, :sw]
                    musq = stt.tile([128, SPAN], F32, tag="musq", name="musq")
                    nc.scalar.activation(
                        musq[0:nr, :sw], mu_t, Act.Square)
                    var = stt.tile([128, SPAN], F32, tag="var", name="var")
                    nc.vector.tensor_tensor(
                        var[0:nr, :sw], ms_t,
                        musq[0:nr, :sw], op=Alu.subtract)
                    sq = musq  # reuse: musq's last reader (var) is done
                    nc.scalar.activation(
                        sq[0:nr, :sw], var[0:nr, :sw], Act.Sqrt,
                        bias=eps_t[0:nr, 0:1], scale=1.0)
                    nc.vector.reciprocal_approx_fast(
                        out=srstd[qs][0:nr, :sw], in_=sq[0:nr, :sw])
                    nc.scalar.activation(
                        srstd_r[qs][0:nr, :sw], srstd[qs][0:nr, :sw],
                        Act.Copy)
                    # pack lhsT prep: [x*rstd_rep ; mu*rstd ; ones]
                    for p in spacks:
                        q0 = 32 * p.quad
                        t_ = xpt[(p.pid, si % 2)]
                        nc.vector.tensor_tensor(
                            t_[p.F32:p.F32 + p.n, :sw], mu_ap(p, q0),
                            srstd[p.qset][q0:q0 + p.n, :sw], op=Alu.mult)
                        rr = rrps.tile([128, SPAN], F32, tag="rr", name="rr")
                        nc.tensor.matmul(
                            rr[0:p.F, :sw],
                            erep[p.qset][q0:q0 + p.n, 0:p.F],
                            srstd_r[p.qset][q0:q0 + p.n, :sw],
                            start=True, stop=True)
                        nc.vector.tensor_tensor(
                            t_[0:p.F, :sw], xin(p.pid, s0, sw),
                            rr[0:p.F, :sw], op=Alu.mult)

            drain_i = 0
            prep_span(0, SET_ORDER)
            for si, (s0, sw) in enumerate(SPANS):
                for ci, c0 in enumerate(range(s0, s0 + sw, CHUNK)):
                    cw = min(CHUNK, s0 + sw - c0)
                    ob = obp.tile([128, OBCOLS], F32, tag="ob", name="ob")
                    for p in PACKS:
                        lhsT = xpt[(p.pid, si % 2)][0:p.K, c0 - s0:c0 - s0 + cw]
                        ob0 = p.k0 * C
                        for (n0, nw, kb0, nb) in _blocks(p):
                            op = outps.tile([128, 512], F32, tag="op", name="op")
                            nc.tensor.matmul(
                                op[0:cw, 0:nw], lhsT,
                                wext[p.pid][:, n0:n0 + nw],
                                start=True, stop=True)
                            eng = DRAIN_PAT[drain_i % len(DRAIN_PAT)]
                            if eng == "a":
                                nc.scalar.activation(
                                    ob[0:cw, ob0 + n0:ob0 + n0 + nw],
                                    op[0:cw, 0:nw], Act.Copy)
                            elif eng == "v":
                                nc.vector.tensor_copy(
                                    ob[0:cw, ob0 + n0:ob0 + n0 + nw],
                                    op[0:cw, 0:nw])
                            else:
                                nc.gpsimd.tensor_copy(
                                    ob[0:cw, ob0 + n0:ob0 + n0 + nw],
                                    op[0:cw, 0:nw])
                            drain_i += 1
                    dst = out_d[:, c0:c0 + cw, :]
                    dst = dst.rearrange("j t c -> t j c")
                    src = ob[0:cw, :].rearrange("t (j c) -> t j c", c=C)
                    nc.sync.dma_start(out=dst, in_=src)
                    # issue next span's prep mid-span in two slices so the
                    # in-order Act/DVE queues never absorb one big burst
                    if si + 1 < len(SPANS):
                        if ci == 1:
                            prep_span(si + 1, ["C", "A"])
                        elif ci == 2:
                            prep_span(si + 1, ["B"])

    nc.compile()
    _CACHE["nc"] = nc
    return nc


# ------------------------------------------------------------------ driver --
def kernel(**inputs):
    from concourse.bass_utils import run_bass_kernel_spmd

    x = np.ascontiguousarray(np.asarray(inputs["x"], np.float32))
    B = x.shape[0]
    assert x.shape == (8, 2, F_BINS, T)
    ext = _host_params(inputs)
    nc = _build()
    in_maps = []
    for b in range(B):
        m = {"x": x[b].reshape(2 * F_BINS, T)}
        m.update(ext)
        in_maps.append(m)
    res = run_bass_kernel_spmd(nc, in_maps, core_ids=list(range(8)))
    out = np.stack([res.results[b]["out"] for b in range(B)], axis=0)
    return out.astype(np.float32, copy=False)


# revision 59
# speedup vs baseline: 1.6817x; 1.0131x over previous
"""BandSplit (BSRNN) Trainium2 kernel.

Math per band k (31 bands over 257 freq bins, groups of band width 3/6/16/27):
  xg = x[b, :, band_bins, t] flattened to d = 2*bw features (torch order:
       bin-major, re/im minor)
  out[b, k, t, :] = LayerNorm_d(xg) @ W_k + b_k          (d -> C=128)

Algebraic refactor used here (per band, per t):
  mu    = mean_d(x),  var = mean_d(x^2) - mu^2,  rstd = rsqrt(var + eps)
  out   = rstd*(x @ Wg) - (mu*rstd)*(u) + bb
  with (host-precomputed)  Wg = gamma*W,  u = sum_d Wg,  bb = b + beta @ W.
So on device, for pack lhsT rows = [x*rstd_rep ; pad ; mu*rstd ; ones ; pad]
and rhs = [Wg blockdiag ; 0 ; -u blockdiag ; bb row ; 0], a single fp32r
matmul per (pack, t-chunk) emits the FINAL output tile (t x (n*128)) in PSUM.

All matmul inputs are float32r (TF32-like rounded fp32): the PE streams
1 column/cycle instead of 4 when the moving free dim >= 256, a 4x speedup
worth ~4e-3 relative error (gate is 2e-2). fp32r matmuls require output
partition base 0, so the per-band stats sums put the band quadrant offset
in selx's COLUMNS and each qset's packs accumulate into one PSUM tile.

Per 128-t chunk the 11 block PSUM tiles drain (Act/DVE rotation -- GPSIMD
cannot touch PSUM) into one wide SBUF tile [128 x 31*128], shipped to DRAM
by a single DMA (t-major, one 512B descriptor per (t, band) run) -- full
360 GB/s with ~1 HWDGE setup per chunk.

Pipeline: span s+1's stats/rstd/lhsT prep is issued one qset per chunk
slot of span s, so the in-order Act/DVE queues absorb small prep bursts
between drain work instead of stalling the store stream at span
boundaries. All DMAs ride the SP queue in consumption order.

Sharding: batch-parallel, core b handles x[b] (B=8 = n_cores).
"""

import numpy as np

T = 3000
C = 128
F_BINS = 257
EPS = 1e-5
GROUPS = [(10, 3), (12, 6), (8, 16), (1, 27)]  # (n_bands, bins_per_band)

SPAN = 512   # stats/prep span (free dim of PSUM bank)
CHUNK = 128  # output t-chunk (PSUM partition dim)
OBCOLS = 31 * C


# ---------------------------------------------------------------- metadata --
class Band:
    def __init__(self, g, i, f0, bw):
        self.g, self.i, self.f0, self.bw = g, i, f0, bw


BANDS = []
_f0 = 0
for _g, (_n, _bw) in enumerate(GROUPS):
    for _i in range(_n):
        BANDS.append(Band(_g, _i, _f0, _bw))
        _f0 += _bw
assert _f0 == F_BINS and len(BANDS) == 31


class Pack:
    def __init__(self, pid, band_ids, qset, quad):
        self.pid = pid
        self.band_ids = list(band_ids)
        self.n = len(self.band_ids)
        self.bw = BANDS[self.band_ids[0]].bw
        self.d = 2 * self.bw
        self.F = self.n * self.d                    # feature rows
        self.F32 = ((self.F + 31) // 32) * 32       # aligned start of mu'' rows
        self.K = self.F32 + 32                      # lhsT partition count
        self.row_ones = self.F32 + self.n
        self.k0 = self.band_ids[0]                  # first global band
        self.f0 = BANDS[self.k0].f0                 # first freq bin
        self.qset = qset                            # 'A' or 'B'
        self.quad = quad                            # quadrant row base /32
        assert self.K <= 128 and self.F32 + self.n + 1 <= self.K


# matmul tile_position bases are limited to {0, 32, 64} (quadrant-3 HW bug),
# so at most 3 packs share a stats/srstd tile set.
PACKS = [
    Pack(0, range(0, 10), 'A', 0),
    Pack(1, range(10, 16), 'A', 1),
    Pack(2, range(16, 22), 'A', 2),
    Pack(3, range(22, 25), 'B', 0),
    Pack(4, range(25, 28), 'B', 1),
    Pack(5, range(28, 30), 'B', 2),
    Pack(6, range(30, 31), 'C', 0),
]
QSETS = "ABC"
EREP_COLS = max(p.F for p in PACKS)  # 96
# emission / load order: C first (one tiny band -> its chain clears the
# in-order Act/DVE queues almost immediately), then A, then B.
SET_ORDER = ["C", "A", "B"]
LOAD_ORDER = [6, 0, 1, 2, 3, 4, 5]

SPANS = [(s0, min(SPAN, T - s0)) for s0 in range(0, T, SPAN)]

# drain engine per block: a=Act (0.83ns/col), v=DVE (1.04ns/col); Pool
# cannot read PSUM. 11 blocks/chunk, widths [512,512,256, 512,256,
# 512,256, 384, 384, 256, 128]; split tuned against the timeline sim.
DRAIN_PAT = ["a", "v", "v", "a", "v", "a", "v", "a", "a", "v", "a"]


def _blocks(p):
    """512-wide column blocks of pack p's (n*128) output."""
    out = []
    for n0 in range(0, p.n * C, 512):
        nw = min(512, p.n * C - n0)
        out.append((n0, nw, p.k0 + n0 // C, nw // C))  # col0, width, band0, nbands
    return out


# ------------------------------------------------------------- host params --
def _host_params(inputs):
    f32 = np.float32
    ext = {}
    for p in PACKS:
        # selx rides as 128 extra columns of wext (rows 0:F) so one DMA
        # loads both. The band quadrant offset lives in selx's COLUMNS
        # (32*quad + j) so the f32r stats matmuls write at output
        # partition base 0 (fp32r + nonzero tile_position col base is
        # rejected by codegen); a set's packs accumulate into one tile.
        wext = np.zeros((p.K, p.n * C + 128), f32)
        selx = wext[0:p.F, p.n * C:]
        F2 = p.F // 2
        for j, k in enumerate(p.band_ids):
            b = BANDS[k]
            W = np.asarray(inputs[f"g{b.g}_W"][b.i], f32)        # (d, C)
            gam = np.asarray(inputs[f"g{b.g}_gamma"][b.i], f32)  # (d,)
            bet = np.asarray(inputs[f"g{b.g}_beta"][b.i], f32)
            bias = np.asarray(inputs[f"g{b.g}_b"][b.i], f32)     # (C,)
            Wg = gam[:, None] * W
            cols = slice(j * C, (j + 1) * C)
            # device row layout is plane-major: row pl*F2 + j*bw + f
            # holds (plane pl, bin f) of band j == torch feature 2f+pl
            for pl in range(2):
                rows = slice(pl * F2 + j * p.bw, pl * F2 + (j + 1) * p.bw)
                wext[rows, cols] = Wg[2 * np.arange(p.bw) + pl]
                selx[rows, 32 * p.quad + j] = 1.0 / p.d
            wext[p.F32 + j, cols] = -Wg.sum(0)
            wext[p.row_ones, cols] = bias + bet @ W
        ext[f"wext{p.pid}"] = wext
    for qs in QSETS:
        erep = np.zeros((128, EREP_COLS), f32)
        for p in PACKS:
            if p.qset != qs:
                continue
            F2 = p.F // 2
            for j in range(p.n):
                for pl in range(2):
                    r0 = pl * F2 + j * p.bw
                    erep[32 * p.quad + j, r0:r0 + p.bw] = 1.0
        ext[f"erep{qs}"] = erep
    return ext


# ------------------------------------------------------------ device build --
_CACHE = {}


def _build():
    if "nc" in _CACHE:
        return _CACHE["nc"]
    import concourse.bacc as bacc
    import concourse.tile as tile
    from concourse import mybir

    Alu = mybir.AluOpType
    Act = mybir.ActivationFunctionType
    F32 = mybir.dt.float32
    F32R = mybir.dt.float32r

    nc = bacc.Bacc("TRN2", target_bir_lowering=False, debug=False, num_devices=8)
    x_d = nc.dram_tensor("x", [2 * F_BINS, T], F32R, kind="ExternalInput")
    out_d = nc.dram_tensor("out", [31, T, C], F32, kind="ExternalOutput")
    wext_d = {p.pid: nc.dram_tensor(f"wext{p.pid}", [p.K, p.n * C + 128], F32R,
                                    kind="ExternalInput") for p in PACKS}
    erep_d = {qs: nc.dram_tensor(f"erep{qs}", [128, EREP_COLS], F32R,
                                 kind="ExternalInput") for qs in QSETS}

    with tile.TileContext(nc) as tc:
        with (
            tc.tile_pool(name="const", bufs=1) as const,
            tc.tile_pool(name="xsqp", bufs=2) as xsqp,
            tc.tile_pool(name="stt", bufs=2) as stt,
            tc.tile_pool(name="obp", bufs=2) as obp,
            tc.tile_pool(name="srp", bufs=1) as srp,
            tc.tile_pool(name="stps", bufs=1, space="PSUM") as stps,
            tc.tile_pool(name="rrps", bufs=1, space="PSUM") as rrps,
            tc.tile_pool(name="outps", bufs=4, space="PSUM") as outps,
        ):
            # ---- resident constants, loaded on the SP queue in the order
            # the compute pipeline consumes them (pack by pack; erep for a
            # qset as soon as all its packs are in flight).
            xin_t = {}
            wext = {}
            selx = {}
            erep = {}
            for p in PACKS:
                xin_t[p.pid] = const.tile([p.F, T], F32R, tag=f"xin{p.pid}", name=f"xin{p.pid}")
                wext[p.pid] = const.tile([p.K, p.n * C + 128], F32R, tag=f"wx{p.pid}", name=f"wx{p.pid}")
                selx[p.pid] = wext[p.pid][0:p.F, p.n * C:p.n * C + 128]
            for qs in QSETS:
                erep[qs] = const.tile([128, EREP_COLS], F32R, tag=f"er{qs}", name=f"er{qs}")

            def xin(pid, s0, sw):
                return xin_t[pid][:, s0:s0 + sw]

            for pid in LOAD_ORDER:
                p = PACKS[pid]
                F2 = p.F // 2
                # SBUF rows are plane-major: [all re bins | all im bins],
                # one contiguous 2D DMA per plane.
                for pl in range(2):
                    s_ = x_d[pl * F_BINS + p.f0:pl * F_BINS + p.f0 + F2, :]
                    d_ = xin_t[p.pid][pl * F2:(pl + 1) * F2, :]
                    nc.sync.dma_start(out=d_, in_=s_)
                # full K rows: the all-zero pad rows must be resident so
                # lhsT pad rows (set to 1.0) multiply against true zeros
                nc.sync.dma_start(out=wext[p.pid][:], in_=wext_d[p.pid][:])
                qs = p.qset
                if pid == max(q.pid for q in PACKS if q.qset == qs):
                    nc.sync.dma_start(out=erep[qs][:], in_=erep_d[qs][:])

            eps_t = const.tile([128, 1], F32, tag="epsc", name="epsc")
            nc.vector.memset(eps_t[:], EPS)

            # persistent double-buffered lhsT tiles (f32r). Rows
            # [floor32(F), K) are filled with 1.0 once: the ones row gets
            # its 1.0, pad rows become finite don't-cares (their wext rows
            # are zero), data rows in range are rewritten every span.
            # memset can't emit fp32r, so fill via fp32 scratch + DVE copy
            # (the copy rounds on write).
            ones_t = const.tile([32, SPAN], F32, tag="ones", name="ones")
            nc.gpsimd.memset(ones_t[:], 1.0)
            xpt = {}
            for p in PACKS:
                ms0 = (p.F // 32) * 32
                for par in range(2):
                    t_ = const.tile([p.K, SPAN], F32R, tag=f"xp{p.pid}_{par}",
                                    name=f"xp{p.pid}_{par}")
                    xpt[(p.pid, par)] = t_
                    for m0 in range(ms0, p.K, 32):
                        nc.vector.tensor_copy(t_[m0:m0 + 32, :], ones_t[:])

            spst = {}  # (si) -> stats tiles, allocated at first set

            def prep_span(si, sets):
                """Stats + rstd + lhsT prep for span si, given qsets only
                (engines: PE stats, Act square/sqrt, DVE sub/recip/x*rr,
                Pool mu*rstd)."""
                s0, sw = SPANS[si]
                # A) per-band sums via PE: mu / meansq at quadrant rows
                if si not in spst:
                    spst[si] = (
                        {qs: stps.tile([128, SPAN], F32, tag="mu",
                                       name=f"mu{qs}", bufs=2) for qs in "ABC"},
                        {qs: stps.tile([128, SPAN], F32, tag="ms",
                                       name=f"ms{qs}", bufs=1) for qs in "ABC"},
                        None,
                        {qs: srp.tile([128, SPAN], F32, tag=f"sr{qs}",
                                      name=f"sr{qs}") for qs in QSETS},
                        {qs: srp.tile([128, SPAN], F32R, tag=f"sq{qs}",
                                      name=f"sq{qs}") for qs in QSETS},
                    )
                    spst.pop(si - 2, None)
                mu, msq, _, srstd, srstd_r = spst[si]

                def mu_ap(p, q0, w=None):
                    w = p.n if w is None else w
                    return mu[p.qset][q0:q0 + w, :sw]

                def msq_ap(p, q0, w=None):
                    w = p.n if w is None else w
                    return msq[p.qset][q0:q0 + w, :sw]

                # per-SET emission (stats -> chain -> pack prep) so each
                # in-order engine queue finishes set qs before touching
                # ops that depend on later-loaded packs.
                for qs in sets:
                    spacks = [p for p in PACKS if p.qset == qs]
                    nr = 1 if qs == "C" else 96  # rows in this set's chain
                    xsqs = {}
                    for p in spacks:
                        xsq = xsqp.tile([128, SPAN], F32R, tag="xsq", name="xsq")
                        xsqs[p.pid] = xsq
                        xin_f = xin(p.pid, s0, sw)
                        nc.gpsimd.tensor_tensor(
                            xsq[0:p.F, :sw], xin_f, xin_f, op=Alu.mult)
                    # per-band sums: the set's packs ACCUMULATE into one
                    # [128, sw] tile (each selx writes only its quadrant
                    # rows via its column offsets; the rest add zeros)
                    np_ = len(spacks)
                    for pi, p in enumerate(spacks):
                        nc.tensor.matmul(
                            mu[qs][:, :sw], selx[p.pid],
                            xin(p.pid, s0, sw),
                            start=(pi == 0), stop=(pi == np_ - 1))
                    for pi, p in enumerate(spacks):
                        nc.tensor.matmul(
                            msq[qs][:, :sw], selx[p.pid],
                            xsqs[p.pid][0:p.F, :sw],
                            start=(pi == 0), stop=(pi == np_ - 1))
                    # rstd = 1/sqrt(msq - mu^2 + eps), batched per set
                    mu_t = mu[qs][0:nr, :sw]
                    ms_t = msq[qs][0:nr, :sw]
                    musq = stt.tile([128, SPAN], F32, tag="musq", name="musq")
                    nc.scalar.activation(
                        musq[0:nr, :sw], mu_t, Act.Square)
                    var = stt.tile([128, SPAN], F32, tag="var", name="var")
                    nc.vector.tensor_tensor(
                        var[0:nr, :sw], ms_t,
                        musq[0:nr, :sw], op=Alu.subtract)
                    sq = musq  # reuse: musq's last reader (var) is done
                    nc.scalar.activation(
                        sq[0:nr, :sw], var[0:nr, :sw], Act.Sqrt,
                        bias=eps_t[0:nr, 0:1], scale=1.0)
                    nc.vector.reciprocal_approx_fast(
                        out=srstd[qs][0:nr, :sw], in_=sq[0:nr, :sw])
                    nc.scalar.activation(
                        srstd_r[qs][0:nr, :sw], srstd[qs][0:nr, :sw],
                        Act.Copy)
                    # pack lhsT prep: [x*rstd_rep ; mu*rstd ; ones]
                    for p in spacks:
                        q0 = 32 * p.quad
                        t_ = xpt[(p.pid, si % 2)]
                        nc.vector.tensor_tensor(
                            t_[p.F32:p.F32 + p.n, :sw], mu_ap(p, q0),
                            srstd[p.qset][q0:q0 + p.n, :sw], op=Alu.mult)
                        rr = rrps.tile([128, SPAN], F32, tag="rr", name="rr")
                        nc.tensor.matmul(
                            rr[0:p.F, :sw],
                            erep[p.qset][q0:q0 + p.n, 0:p.F],
                            srstd_r[p.qset][q0:q0 + p.n, :sw],
                            start=True, stop=True)
                        nc.vector.tensor_tensor(
                            t_[0:p.F, :sw], xin(p.pid, s0, sw),
                            rr[0:p.F, :sw], op=Alu.mult)

            drain_i = 0
            prep_span(0, SET_ORDER)
            for si, (s0, sw) in enumerate(SPANS):
                for ci, c0 in enumerate(range(s0, s0 + sw, CHUNK)):
                    cw = min(CHUNK, s0 + sw - c0)
                    ob = obp.tile([128, OBCOLS], F32, tag="ob", name="ob")
                    for p in PACKS:
                        lhsT = xpt[(p.pid, si % 2)][0:p.K, c0 - s0:c0 - s0 + cw]
                        ob0 = p.k0 * C
                        for (n0, nw, kb0, nb) in _blocks(p):
                            op = outps.tile([128, 512], F32, tag="op", name="op")
                            nc.tensor.matmul(
                                op[0:cw, 0:nw], lhsT,
                                wext[p.pid][:, n0:n0 + nw],
                                start=True, stop=True)
                            eng = DRAIN_PAT[drain_i % len(DRAIN_PAT)]
                            if eng == "a":
                                nc.scalar.activation(
                                    ob[0:cw, ob0 + n0:ob0 + n0 + nw],
                                    op[0:cw, 0:nw], Act.Copy)
                            elif eng == "v":
                                nc.vector.tensor_copy(
                                    ob[0:cw, ob0 + n0:ob0 + n0 + nw],
                                    op[0:cw, 0:nw])
                            else:
                                nc.gpsimd.tensor_copy(
                                    ob[0:cw, ob0 + n0:ob0 + n0 + nw],
                                    op[0:cw, 0:nw])
                            drain_i += 1
                    if si == 0:
                        # span 0: ship bands 0..21 (packs 0-2, whose prep
                        # finishes ~10us before the B set's) as soon as
                        # their drains land, filling the DMA idle window
                        # between the last input and the first full chunk
                        dstA = out_d[0:22, c0:c0 + cw, :]
                        dstA = dstA.rearrange("j t c -> t j c")
                        srcA = ob[0:cw, 0:22 * C].rearrange(
                            "t (j c) -> t j c", c=C)
                        nc.sync.dma_start(out=dstA, in_=srcA)
                        dstB = out_d[22:31, c0:c0 + cw, :]
                        dstB = dstB.rearrange("j t c -> t j c")
                        srcB = ob[0:cw, 22 * C:].rearrange(
                            "t (j c) -> t j c", c=C)
                        nc.sync.dma_start(out=dstB, in_=srcB)
                    else:
                        dst = out_d[:, c0:c0 + cw, :]
                        dst = dst.rearrange("j t c -> t j c")
                        src = ob[0:cw, :].rearrange("t (j c) -> t j c", c=C)
                        nc.sync.dma_start(out=dst, in_=src)
                    # issue next span's prep mid-span in two slices so the
                    # in-order Act/DVE queues never absorb one big burst
                    if si + 1 < len(SPANS):
                        if ci == 1:
                            prep_span(si + 1, ["C", "A"])
                        elif ci == 2:
                            prep_span(si + 1, ["B"])

    nc.compile()
    _CACHE["nc"] = nc
    return nc


# ------------------------------------------------------------------ driver --
def kernel(**inputs):
    from concourse.bass_utils import run_bass_kernel_spmd

    x = np.ascontiguousarray(np.asarray(inputs["x"], np.float32))
    B = x.shape[0]
    assert x.shape == (8, 2, F_BINS, T)
    ext = _host_params(inputs)
    nc = _build()
    in_maps = []
    for b in range(B):
        m = {"x": x[b].reshape(2 * F_BINS, T)}
        m.update(ext)
        in_maps.append(m)
    res = run_bass_kernel_spmd(nc, in_maps, core_ids=list(range(8)))
    out = np.stack([res.results[b]["out"] for b in range(B)], axis=0)
    return out.astype(np.float32, copy=False)


# revision 62
# speedup vs baseline: 1.6902x; 1.0050x over previous
"""BandSplit (BSRNN) Trainium2 kernel.

Math per band k (31 bands over 257 freq bins, groups of band width 3/6/16/27):
  xg = x[b, :, band_bins, t] flattened to d = 2*bw features (torch order:
       bin-major, re/im minor)
  out[b, k, t, :] = LayerNorm_d(xg) @ W_k + b_k          (d -> C=128)

Algebraic refactor used here (per band, per t):
  mu    = mean_d(x),  var = mean_d(x^2) - mu^2,  rstd = rsqrt(var + eps)
  out   = rstd*(x @ Wg) - (mu*rstd)*(u) + bb
  with (host-precomputed)  Wg = gamma*W,  u = sum_d Wg,  bb = b + beta @ W.
So on device, for pack lhsT rows = [x*rstd_rep ; pad ; mu*rstd ; ones ; pad]
and rhs = [Wg blockdiag ; 0 ; -u blockdiag ; bb row ; 0], a single fp32r
matmul per (pack, t-chunk) emits the FINAL output tile (t x (n*128)) in PSUM.

All matmul inputs are float32r (TF32-like rounded fp32): the PE streams
1 column/cycle instead of 4 when the moving free dim >= 256, a 4x speedup
worth ~4e-3 relative error (gate is 2e-2). fp32r matmuls require output
partition base 0, so the per-band stats sums put the band quadrant offset
in selx's COLUMNS and each qset's packs accumulate into one PSUM tile.

Per 128-t chunk the 11 block PSUM tiles drain (Act/DVE rotation -- GPSIMD
cannot touch PSUM) into one wide SBUF tile [128 x 31*128], shipped to DRAM
by a single DMA (t-major, one 512B descriptor per (t, band) run) -- full
360 GB/s with ~1 HWDGE setup per chunk.

Pipeline: span s+1's stats/rstd/lhsT prep is issued one qset per chunk
slot of span s, so the in-order Act/DVE queues absorb small prep bursts
between drain work instead of stalling the store stream at span
boundaries. All DMAs ride the SP queue in consumption order.

Sharding: batch-parallel, core b handles x[b] (B=8 = n_cores).
"""

import numpy as np

T = 3000
C = 128
F_BINS = 257
EPS = 1e-5
GROUPS = [(10, 3), (12, 6), (8, 16), (1, 27)]  # (n_bands, bins_per_band)

SPAN = 512   # stats/prep span (free dim of PSUM bank)
CHUNK = 128  # output t-chunk (PSUM partition dim)
OBCOLS = 31 * C


# ---------------------------------------------------------------- metadata --
class Band:
    def __init__(self, g, i, f0, bw):
        self.g, self.i, self.f0, self.bw = g, i, f0, bw


BANDS = []
_f0 = 0
for _g, (_n, _bw) in enumerate(GROUPS):
    for _i in range(_n):
        BANDS.append(Band(_g, _i, _f0, _bw))
        _f0 += _bw
assert _f0 == F_BINS and len(BANDS) == 31


class Pack:
    def __init__(self, pid, band_ids, qset, quad):
        self.pid = pid
        self.band_ids = list(band_ids)
        self.n = len(self.band_ids)
        self.bw = BANDS[self.band_ids[0]].bw
        self.d = 2 * self.bw
        self.F = self.n * self.d                    # feature rows
        self.F32 = ((self.F + 31) // 32) * 32       # aligned start of mu'' rows
        self.K = self.F32 + 32                      # lhsT partition count
        self.row_ones = self.F32 + self.n
        self.k0 = self.band_ids[0]                  # first global band
        self.f0 = BANDS[self.k0].f0                 # first freq bin
        self.qset = qset                            # 'A' or 'B'
        self.quad = quad                            # quadrant row base /32
        assert self.K <= 128 and self.F32 + self.n + 1 <= self.K


# matmul tile_position bases are limited to {0, 32, 64} (quadrant-3 HW bug),
# so at most 3 packs share a stats/srstd tile set.
PACKS = [
    Pack(0, range(0, 10), 'A', 0),
    Pack(1, range(10, 16), 'A', 1),
    Pack(2, range(16, 22), 'A', 2),
    Pack(3, range(22, 25), 'B', 0),
    Pack(4, range(25, 28), 'B', 1),
    Pack(5, range(28, 30), 'B', 2),
    Pack(6, range(30, 31), 'C', 0),
]
QSETS = "ABC"
EREP_COLS = max(p.F for p in PACKS)  # 96
# emission / load order: C first (one tiny band -> its chain clears the
# in-order Act/DVE queues almost immediately), then A, then B.
SET_ORDER = ["C", "A", "B"]
LOAD_ORDER = [6, 0, 1, 2, 3, 4, 5]

SPANS = [(s0, min(SPAN, T - s0)) for s0 in range(0, T, SPAN)]

# drain engine per block: a=Act (0.83ns/col), v=DVE (1.04ns/col); Pool
# cannot read PSUM. 11 blocks/chunk, widths [512,512,256, 512,256,
# 512,256, 384, 384, 256, 128]; split tuned against the timeline sim.
DRAIN_PAT = ["a", "v", "v", "a", "v", "a", "v", "a", "a", "v", "a"]


def _blocks(p):
    """512-wide column blocks of pack p's (n*128) output."""
    out = []
    for n0 in range(0, p.n * C, 512):
        nw = min(512, p.n * C - n0)
        out.append((n0, nw, p.k0 + n0 // C, nw // C))  # col0, width, band0, nbands
    return out


# ------------------------------------------------------------- host params --
def _host_params(inputs):
    f32 = np.float32
    ext = {}
    for p in PACKS:
        # selx rides as 128 extra columns of wext (rows 0:F) so one DMA
        # loads both. The band quadrant offset lives in selx's COLUMNS
        # (32*quad + j) so the f32r stats matmuls write at output
        # partition base 0 (fp32r + nonzero tile_position col base is
        # rejected by codegen); a set's packs accumulate into one tile.
        wext = np.zeros((p.K, p.n * C + 128), f32)
        selx = wext[0:p.F, p.n * C:]
        F2 = p.F // 2
        for j, k in enumerate(p.band_ids):
            b = BANDS[k]
            W = np.asarray(inputs[f"g{b.g}_W"][b.i], f32)        # (d, C)
            gam = np.asarray(inputs[f"g{b.g}_gamma"][b.i], f32)  # (d,)
            bet = np.asarray(inputs[f"g{b.g}_beta"][b.i], f32)
            bias = np.asarray(inputs[f"g{b.g}_b"][b.i], f32)     # (C,)
            Wg = gam[:, None] * W
            cols = slice(j * C, (j + 1) * C)
            # device row layout is plane-major: row pl*F2 + j*bw + f
            # holds (plane pl, bin f) of band j == torch feature 2f+pl
            for pl in range(2):
                rows = slice(pl * F2 + j * p.bw, pl * F2 + (j + 1) * p.bw)
                wext[rows, cols] = Wg[2 * np.arange(p.bw) + pl]
                selx[rows, 32 * p.quad + j] = 1.0 / p.d
            wext[p.F32 + j, cols] = -Wg.sum(0)
            wext[p.row_ones, cols] = bias + bet @ W
        ext[f"wext{p.pid}"] = wext
    for qs in QSETS:
        erep = np.zeros((128, EREP_COLS), f32)
        for p in PACKS:
            if p.qset != qs:
                continue
            F2 = p.F // 2
            for j in range(p.n):
                for pl in range(2):
                    r0 = pl * F2 + j * p.bw
                    erep[32 * p.quad + j, r0:r0 + p.bw] = 1.0
        ext[f"erep{qs}"] = erep
    return ext


# ------------------------------------------------------------ device build --
_CACHE = {}


def _build():
    if "nc" in _CACHE:
        return _CACHE["nc"]
    import concourse.bacc as bacc
    import concourse.tile as tile
    from concourse import mybir

    Alu = mybir.AluOpType
    Act = mybir.ActivationFunctionType
    F32 = mybir.dt.float32
    F32R = mybir.dt.float32r

    nc = bacc.Bacc("TRN2", target_bir_lowering=False, debug=False, num_devices=8)
    x_d = nc.dram_tensor("x", [2 * F_BINS, T], F32R, kind="ExternalInput")
    out_d = nc.dram_tensor("out", [31, T, C], F32, kind="ExternalOutput")
    wext_d = {p.pid: nc.dram_tensor(f"wext{p.pid}", [p.K, p.n * C + 128], F32R,
                                    kind="ExternalInput") for p in PACKS}
    erep_d = {qs: nc.dram_tensor(f"erep{qs}", [128, EREP_COLS], F32R,
                                 kind="ExternalInput") for qs in QSETS}

    with tile.TileContext(nc) as tc:
        with (
            tc.tile_pool(name="const", bufs=1) as const,
            tc.tile_pool(name="xsqp", bufs=3) as xsqp,
            tc.tile_pool(name="stt", bufs=3) as stt,
            tc.tile_pool(name="obp", bufs=2) as obp,
            tc.tile_pool(name="srp", bufs=1) as srp,
            tc.tile_pool(name="stps", bufs=1, space="PSUM") as stps,
            tc.tile_pool(name="rrps", bufs=1, space="PSUM") as rrps,
            tc.tile_pool(name="outps", bufs=4, space="PSUM") as outps,
        ):
            # ---- resident constants, loaded on the SP queue in the order
            # the compute pipeline consumes them (pack by pack; erep for a
            # qset as soon as all its packs are in flight).
            xin_t = {}
            wext = {}
            selx = {}
            erep = {}
            for p in PACKS:
                xin_t[p.pid] = const.tile([p.F, T], F32R, tag=f"xin{p.pid}", name=f"xin{p.pid}")
                wext[p.pid] = const.tile([p.K, p.n * C + 128], F32R, tag=f"wx{p.pid}", name=f"wx{p.pid}")
                selx[p.pid] = wext[p.pid][0:p.F, p.n * C:p.n * C + 128]
            for qs in QSETS:
                erep[qs] = const.tile([128, EREP_COLS], F32R, tag=f"er{qs}", name=f"er{qs}")

            def xin(pid, s0, sw):
                return xin_t[pid][:, s0:s0 + sw]

            for pid in LOAD_ORDER:
                p = PACKS[pid]
                F2 = p.F // 2
                # SBUF rows are plane-major: [all re bins | all im bins],
                # one contiguous 2D DMA per plane.
                for pl in range(2):
                    s_ = x_d[pl * F_BINS + p.f0:pl * F_BINS + p.f0 + F2, :]
                    d_ = xin_t[p.pid][pl * F2:(pl + 1) * F2, :]
                    nc.sync.dma_start(out=d_, in_=s_)
                # full K rows: the all-zero pad rows must be resident so
                # lhsT pad rows (set to 1.0) multiply against true zeros
                nc.sync.dma_start(out=wext[p.pid][:], in_=wext_d[p.pid][:])
                qs = p.qset
                if pid == max(q.pid for q in PACKS if q.qset == qs):
                    nc.sync.dma_start(out=erep[qs][:], in_=erep_d[qs][:])

            eps_t = const.tile([128, 1], F32, tag="epsc", name="epsc")
            nc.vector.memset(eps_t[:], EPS)

            # persistent double-buffered lhsT tiles (f32r). Rows
            # [floor32(F), K) are filled with 1.0 once: the ones row gets
            # its 1.0, pad rows become finite don't-cares (their wext rows
            # are zero), data rows in range are rewritten every span.
            # memset can't emit fp32r, so fill via fp32 scratch + DVE copy
            # (the copy rounds on write).
            ones_t = const.tile([32, SPAN], F32, tag="ones", name="ones")
            nc.gpsimd.memset(ones_t[:], 1.0)
            xpt = {}
            for p in PACKS:
                ms0 = (p.F // 32) * 32
                for par in range(2):
                    t_ = const.tile([p.K, SPAN], F32R, tag=f"xp{p.pid}_{par}",
                                    name=f"xp{p.pid}_{par}")
                    xpt[(p.pid, par)] = t_
                    for m0 in range(ms0, p.K, 32):
                        nc.vector.tensor_copy(t_[m0:m0 + 32, :], ones_t[:])

            spst = {}  # (si) -> stats tiles, allocated at first set

            def prep_span(si, sets):
                """Stats + rstd + lhsT prep for span si, given qsets only
                (engines: PE stats, Act square/sqrt, DVE sub/recip/x*rr,
                Pool mu*rstd)."""
                s0, sw = SPANS[si]
                # A) per-band sums via PE: mu / meansq at quadrant rows
                if si not in spst:
                    spst[si] = (
                        {qs: stps.tile([128, SPAN], F32, tag="mu",
                                       name=f"mu{qs}", bufs=2) for qs in "ABC"},
                        {qs: stps.tile([128, SPAN], F32, tag="ms",
                                       name=f"ms{qs}", bufs=1) for qs in "ABC"},
                        None,
                        {qs: srp.tile([128, SPAN], F32, tag=f"sr{qs}",
                                      name=f"sr{qs}") for qs in QSETS},
                        {qs: srp.tile([128, SPAN], F32R, tag=f"sq{qs}",
                                      name=f"sq{qs}") for qs in QSETS},
                    )
                    spst.pop(si - 2, None)
                mu, msq, _, srstd, srstd_r = spst[si]

                def mu_ap(p, q0, w=None):
                    w = p.n if w is None else w
                    return mu[p.qset][q0:q0 + w, :sw]

                def msq_ap(p, q0, w=None):
                    w = p.n if w is None else w
                    return msq[p.qset][q0:q0 + w, :sw]

                # per-SET emission (stats -> chain -> pack prep) so each
                # in-order engine queue finishes set qs before touching
                # ops that depend on later-loaded packs.
                for qs in sets:
                    spacks = [p for p in PACKS if p.qset == qs]
                    nr = 1 if qs == "C" else 96  # rows in this set's chain
                    xsqs = {}
                    for p in spacks:
                        xsq = xsqp.tile([128, SPAN], F32R, tag="xsq", name="xsq")
                        xsqs[p.pid] = xsq
                        xin_f = xin(p.pid, s0, sw)
                        nc.gpsimd.tensor_tensor(
                            xsq[0:p.F, :sw], xin_f, xin_f, op=Alu.mult)
                    # per-band sums: the set's packs ACCUMULATE into one
                    # [128, sw] tile (each selx writes only its quadrant
                    # rows via its column offsets; the rest add zeros)
                    np_ = len(spacks)
                    for pi, p in enumerate(spacks):
                        nc.tensor.matmul(
                            mu[qs][:, :sw], selx[p.pid],
                            xin(p.pid, s0, sw),
                            start=(pi == 0), stop=(pi == np_ - 1))
                    for pi, p in enumerate(spacks):
                        nc.tensor.matmul(
                            msq[qs][:, :sw], selx[p.pid],
                            xsqs[p.pid][0:p.F, :sw],
                            start=(pi == 0), stop=(pi == np_ - 1))
                    # rstd = 1/sqrt(msq - mu^2 + eps), batched per set
                    mu_t = mu[qs][0:nr, :sw]
                    ms_t = msq[qs][0:nr, :sw]
                    musq = stt.tile([128, SPAN], F32, tag="musq", name="musq")
                    nc.scalar.activation(
                        musq[0:nr, :sw], mu_t, Act.Square)
                    var = stt.tile([128, SPAN], F32, tag="var", name="var")
                    nc.vector.tensor_tensor(
                        var[0:nr, :sw], ms_t,
                        musq[0:nr, :sw], op=Alu.subtract)
                    sq = musq  # reuse: musq's last reader (var) is done
                    nc.scalar.activation(
                        sq[0:nr, :sw], var[0:nr, :sw], Act.Sqrt,
                        bias=eps_t[0:nr, 0:1], scale=1.0)
                    nc.vector.reciprocal_approx_fast(
                        out=srstd[qs][0:nr, :sw], in_=sq[0:nr, :sw])
                    nc.scalar.activation(
                        srstd_r[qs][0:nr, :sw], srstd[qs][0:nr, :sw],
                        Act.Copy)
                    # pack lhsT prep: [x*rstd_rep ; mu*rstd ; ones]
                    for p in spacks:
                        q0 = 32 * p.quad
                        t_ = xpt[(p.pid, si % 2)]
                        nc.vector.tensor_tensor(
                            t_[p.F32:p.F32 + p.n, :sw], mu_ap(p, q0),
                            srstd[p.qset][q0:q0 + p.n, :sw], op=Alu.mult)
                        rr = rrps.tile([128, SPAN], F32, tag="rr", name="rr")
                        nc.tensor.matmul(
                            rr[0:p.F, :sw],
                            erep[p.qset][q0:q0 + p.n, 0:p.F],
                            srstd_r[p.qset][q0:q0 + p.n, :sw],
                            start=True, stop=True)
                        nc.vector.tensor_tensor(
                            t_[0:p.F, :sw], xin(p.pid, s0, sw),
                            rr[0:p.F, :sw], op=Alu.mult)

            drain_i = 0
            prep_span(0, SET_ORDER)
            for si, (s0, sw) in enumerate(SPANS):
                for ci, c0 in enumerate(range(s0, s0 + sw, CHUNK)):
                    cw = min(CHUNK, s0 + sw - c0)
                    ob = obp.tile([128, OBCOLS], F32, tag="ob", name="ob")
                    for p in PACKS:
                        lhsT = xpt[(p.pid, si % 2)][0:p.K, c0 - s0:c0 - s0 + cw]
                        ob0 = p.k0 * C
                        for (n0, nw, kb0, nb) in _blocks(p):
                            op = outps.tile([128, 512], F32, tag="op", name="op")
                            nc.tensor.matmul(
                                op[0:cw, 0:nw], lhsT,
                                wext[p.pid][:, n0:n0 + nw],
                                start=True, stop=True)
                            eng = DRAIN_PAT[drain_i % len(DRAIN_PAT)]
                            if eng == "a":
                                nc.scalar.activation(
                                    ob[0:cw, ob0 + n0:ob0 + n0 + nw],
                                    op[0:cw, 0:nw], Act.Copy)
                            elif eng == "v":
                                nc.vector.tensor_copy(
                                    ob[0:cw, ob0 + n0:ob0 + n0 + nw],
                                    op[0:cw, 0:nw])
                            else:
                                nc.gpsimd.tensor_copy(
                                    ob[0:cw, ob0 + n0:ob0 + n0 + nw],
                                    op[0:cw, 0:nw])
                            drain_i += 1
                    if si == 0:
                        # span 0: ship bands 0..21 (packs 0-2, whose prep
                        # finishes ~10us before the B set's) as soon as
                        # their drains land, filling the DMA idle window
                        # between the last input and the first full chunk
                        dstA = out_d[0:22, c0:c0 + cw, :]
                        dstA = dstA.rearrange("j t c -> t j c")
                        srcA = ob[0:cw, 0:22 * C].rearrange(
                            "t (j c) -> t j c", c=C)
                        nc.sync.dma_start(out=dstA, in_=srcA)
                        dstB = out_d[22:31, c0:c0 + cw, :]
                        dstB = dstB.rearrange("j t c -> t j c")
                        srcB = ob[0:cw, 22 * C:].rearrange(
                            "t (j c) -> t j c", c=C)
                        nc.sync.dma_start(out=dstB, in_=srcB)
                    else:
                        dst = out_d[:, c0:c0 + cw, :]
                        dst = dst.rearrange("j t c -> t j c")
                        src = ob[0:cw, :].rearrange("t (j c) -> t j c", c=C)
                        nc.sync.dma_start(out=dst, in_=src)
                    # issue next span's prep mid-span in two slices so the
                    # in-order Act/DVE queues never absorb one big burst
                    if si + 1 < len(SPANS):
                        if ci == 1:
                            prep_span(si + 1, ["C", "A"])
                        elif ci == 2:
                            prep_span(si + 1, ["B"])

    nc.compile()
    _CACHE["nc"] = nc
    return nc


# ------------------------------------------------------------------ driver --
def kernel(**inputs):
    from concourse.bass_utils import run_bass_kernel_spmd

    x = np.ascontiguousarray(np.asarray(inputs["x"], np.float32))
    B = x.shape[0]
    assert x.shape == (8, 2, F_BINS, T)
    ext = _host_params(inputs)
    nc = _build()
    in_maps = []
    for b in range(B):
        m = {"x": x[b].reshape(2 * F_BINS, T)}
        m.update(ext)
        in_maps.append(m)
    res = run_bass_kernel_spmd(nc, in_maps, core_ids=list(range(8)))
    out = np.stack([res.results[b]["out"] for b in range(B)], axis=0)
    return out.astype(np.float32, copy=False)
